# revision 1
# baseline (speedup 1.0000x reference)
"""nn_CoMet Trainium2 kernel.

Math (per batch element s in R^16):
  MLP: h1 = logsig(s@W1.T); h2 = h1 + logsig(h1@W2.T); h3 = h2 + logsig(h2@W3.T)
       nnout = h3@W4.T ; d = nnout[:16]; com-rows C = W4[16:20]
  J = d com/d s  (reverse mode through the MLP, 4 rows)
  out = d - J^T (J J^T)^{-1} J d      (== the QR-based projection in the reference)

Kernel design (per core, batch 32768, hidden-major layout [hidden on partitions,
elements on free axis], tiles of 512 elements):
  - forward: a1 in f32r, a2/a3/dt in bf16; activations via the natural_log_exp
    table only: e=Exp(-a), lg=Ln(e+1) (= -logsig(a) = hm increment),
    r=Exp(-lg) (= sigmoid(a)); sg = 1-r (DVE 4x tensor_scalar) for layers 1,2.
  - backward: layer 3 uses r3 directly as moving operand with stationary
    -diag(C_i)W3; the (1-r3) complement and the residual C_i both fold into a
    per-partition bias B_i = C_i + C_i@W3 that scalar_tensor_tensor applies
    when reading PSUM: v2s2 = (v+B)*sg2, v1s1 = (v+B)*sg1, split per hidden
    half across DVE and Pool.  U accumulates into the same PSUM bank.
  - J and d produced transposed (element-major); Gram products batched with
    broadcast APs into a single packed tile + one reduce; 4x4 normal-equation
    solve batched over 8 supertiles.
"""

import numpy as np
import ml_dtypes

import concourse.bass as bass
import concourse.mybir as mybir
import concourse.tile as tile

NCORES = 8
B = 262144
BP = B // NCORES          # 32768 per core
NS = 16
NH = 256
NCOM = 4
TN = 512                  # elements per pipeline tile
NTILES = BP // TN         # 64
SUP = 2                   # tiles per supertile (postproc unit) -> 1024 elems
GRPS = SUP * TN // 128    # 8 groups of 128 elems per supertile
SOLVE_SUPS = 8            # supertiles per batched solve -> 8192 elems
SROWS = SOLVE_SUPS * GRPS # 64
ASC = 128.0               # backward fp8 scale (M3, B) ; removed via sg1

F32 = mybir.dt.float32
F32R = mybir.dt.float32r
BF16 = mybir.dt.bfloat16
F8E4 = mybir.dt.float8e4
AF = mybir.ActivationFunctionType
OP = mybir.AluOpType
AX = mybir.AxisListType

_cache = {}


# ---------------------------------------------------------------- drain patch
def _patch_tile_drain():
    """walrus in this container rejects >1 sem wait on the TileContext final
    drain ("Too many sync wait commands"); split the waits across several
    drain instructions (1 wait each)."""
    if getattr(tile.TileContext, "_comet_patched", False):
        return
    from concourse.vector_clock import ScopedClock

    def _drain_and_barrier(self, tick_clock, wait_clock):
        nc = self.nc
        drain_inst = nc.sync.drain()
        wait_clock.add_sem_waits(
            drain_inst.ins, ScopedClock({None: tick_clock.global_clock})
        )
        si = drain_inst.ins.sync_info
        waits = list(si.on_wait) if si is not None and si.on_wait else []
        if len(waits) > 1:
            si.on_wait = waits[:1]
            for w in waits[1:]:
                extra = nc.sync.drain()
                esi = extra.ins.sync_info
                if esi is None:
                    import bass_rust
                    extra.ins.sync_info = bass_rust.SyncInfo(
                        on_wait=[w], on_update=[]
                    )
                else:
                    esi.on_wait = list(esi.on_wait or []) + [w]
        nc.all_engine_barrier()
        assert self.sems is not None
        popped = nc._tile_sem_poison_stack.pop()
        assert popped is self._sem_poison
        nc.clear_and_free_semaphores(list(self.sems.allocated().values()))
        nc.all_engine_barrier()

    tile.TileContext._drain_and_barrier = _drain_and_barrier
    tile.TileContext._comet_patched = True


def _split_multi_waits(nc):
    """This container's walrus rejects instructions with more than one sync
    wait command.  Hoist extra waits onto injected same-engine NOPs placed
    immediately before the offending instruction."""
    import bass_rust

    for f in nc.m.functions:
        for b in f.blocks:
            insts = list(b.instructions)
            out, dirty = [], False
            for inst in insts:
                si = inst.sync_info
                waits = list(si.on_wait) if si is not None and si.on_wait else []
                if len(waits) > 1:
                    dirty = True
                    for k, wx in enumerate(waits[:-1]):
                        nop = mybir.InstNoOp(name=f"{inst.name}-ws{k}")
                        nop.engine = inst.engine
                        nop.sync_info = bass_rust.SyncInfo(
                            on_wait=[wx], on_update=[])
                        out.append(nop)
                    si.on_wait = waits[-1:]
                out.append(inst)
            if dirty:
                b.instructions = out


# ---------------------------------------------------------------- host prep
def _prep_weights(W1, W2, W3, W4):
    W1 = np.asarray(W1, np.float32)
    W2 = np.asarray(W2, np.float32)
    W3 = np.asarray(W3, np.float32)
    W4 = np.asarray(W4, np.float32)
    W4d, C = W4[:NS], W4[NS:]
    bf = ml_dtypes.bfloat16

    def khalf(a, m):   # [256, m] -> [128, 2, m]
        return np.ascontiguousarray(a.reshape(2, 128, m).transpose(1, 0, 2))

    f8 = ml_dtypes.float8_e4m3
    L1 = np.ascontiguousarray(W1.T)                       # [16,256] lhsT for a1
    L2 = khalf(-W2.T, NH).astype(bf)                      # [128,2,256]
    L3 = khalf(-W3.T, NH).astype(bf)
    L4 = khalf(-W4d.T, NS).astype(bf)                     # [128,2,16]
    # stage_a stationary: -diag(C_i)W3 (moving operand is r3 = sigmoid(a3)),
    # scaled by ASC and quantized to fp8 e4m3 for the DoubleRow matmul; the
    # scale is removed by the 1/ASC factor folded into sg1.
    M3 = np.stack([-C[i][:, None] * W3 for i in range(4)])  # [4,256,256]
    M3 = np.ascontiguousarray(
        (ASC * M3).reshape(4, 2, 128, NH).transpose(2, 1, 0, 3)  # [128,2,4,256]
    ).astype(f8)
    W2b = khalf(W2, NH).astype(bf)                        # [128,2,256] lhsT for U
    W1b = khalf(W1, NS).astype(bf)                        # [128,2,16]  rhs for Jt
    # bias B_i = C_i + C_i@W3 (folds the residual C_i and the (1-r3)
    # complement of the layer-3 backward), injected into PSUM once per i by a
    # K=1 ones-matmul during stage_a; it then serves both backward stages.
    Bm = ASC * (C + C @ W3)                               # [4, 256]
    Cb = np.ascontiguousarray(Bm[:, None, :]).transpose(1, 0, 2)  # [1,4,256]
    return {
        "L1": L1, "L2": L2, "L3": L3, "L4": L4,
        "M3": M3, "W2b": W2b, "W1b": W1b, "Cb": Cb.astype(bf),
    }


# ---------------------------------------------------------------- build
def _build(ntiles=NTILES, a_bufs=1, v_bufs=2):
    _patch_tile_drain()
    nc = bass.Bass()

    zT = nc.dram_tensor("zT", [NS, BP], F32R, kind="ExternalInput")
    dL1 = nc.dram_tensor("L1", [NS, NH], F32R, kind="ExternalInput")
    dL2 = nc.dram_tensor("L2", [128, 2, NH], BF16, kind="ExternalInput")
    dL3 = nc.dram_tensor("L3", [128, 2, NH], BF16, kind="ExternalInput")
    dL4 = nc.dram_tensor("L4", [128, 2, NS], BF16, kind="ExternalInput")
    dM3 = nc.dram_tensor("M3", [128, 2, 4, NH], F8E4, kind="ExternalInput")
    dW2b = nc.dram_tensor("W2b", [128, 2, NH], BF16, kind="ExternalInput")
    dW1b = nc.dram_tensor("W1b", [128, 2, NS], BF16, kind="ExternalInput")
    dCb = nc.dram_tensor("Cb", [1, 4, NH], BF16, kind="ExternalInput")
    out_d = nc.dram_tensor("out", [BP, NS], F32, kind="ExternalOutput")

    from contextlib import ExitStack
    with tile.TileContext(nc) as tc, ExitStack() as ctx:
        wpool = ctx.enter_context(tc.tile_pool(name="w", bufs=1))
        sb = ctx.enter_context(tc.tile_pool(name="sb", bufs=3))
        sbe = ctx.enter_context(tc.tile_pool(name="sbe", bufs=6))
        sbv = ctx.enter_context(tc.tile_pool(name="sbv", bufs=6))
        sbz = ctx.enter_context(tc.tile_pool(name="sbz", bufs=6))
        jpool = ctx.enter_context(tc.tile_pool(name="jp", bufs=2))
        mpool = ctx.enter_context(tc.tile_pool(name="mp", bufs=2))
        aps_pool = ctx.enter_context(tc.tile_pool(name="aps", bufs=a_bufs, space="PSUM"))
        vps_pool = ctx.enter_context(tc.tile_pool(name="vps", bufs=v_bufs, space="PSUM"))
        tps_pool = ctx.enter_context(tc.tile_pool(name="tps", bufs=1, space="PSUM"))

        # ---- load constants
        L1s = wpool.tile([NS, NH], F32R)
        nc.sync.dma_start(L1s[:], dL1[:])
        L2s = wpool.tile([128, 2, NH], BF16)
        nc.sync.dma_start(L2s[:], dL2[:])
        L3s = wpool.tile([128, 2, NH], BF16)
        nc.sync.dma_start(L3s[:], dL3[:])
        L4s = wpool.tile([128, 2, NS], BF16)
        nc.sync.dma_start(L4s[:], dL4[:])
        M3s = wpool.tile([128, 2, 4, NH], F8E4)
        nc.sync.dma_start(M3s[:], dM3[:])
        W2s = wpool.tile([128, 2, NH], BF16)
        nc.sync.dma_start(W2s[:], dW2b[:])
        W1s = wpool.tile([128, 2, NS], BF16)
        nc.sync.dma_start(W1s[:], dW1b[:])
        Cbs = wpool.tile([1, 4, NH], BF16)
        nc.sync.dma_start(Cbs[:], dCb[:])
        ones = wpool.tile([1, TN], BF16)
        nc.vector.memset(ones[:], 1.0)

        # ---- hand-scheduled software pipeline.
        # Iteration k emits, in an order matched to each engine's ready-times:
        #   forward(tf=k) spine ops, backward(tb=k-1) stages, deferred
        #   postproc (prods/reduces) for the supertile finished at k-3, and
        #   slices of the deferred solve/combine queue on the Pool engine.
        from collections import deque

        st = {}        # per-tile forward state
        sup = {}       # per-supertile PSUM accumulators
        grp = {"nsup": 0}
        pool_q = deque()   # (cost_ns, min_iter, closure) deferred solve/combine
        cur_k = [0]

        def drain_pool_q(budget):
            while pool_q and budget > 0:
                cost, mink, fn = pool_q[0]
                if mink > cur_k[0]:
                    break
                pool_q.popleft()
                fn()
                budget -= cost

        def emit_fwd(t, part):
            S = st.setdefault(t, {})
            w, s = t % SUP, t // SUP
            if part == 0:      # dma + a1
                if w == 0:
                    sup[s] = {
                        "jt": tps_pool.tile([128, GRPS, 4 * NS], F32, tag="jtps", name="jt_ps"),
                        "dt": tps_pool.tile([128, GRPS, NS], F32, tag="dtps", name="dt_ps"),
                    }
                zt = sbz.tile([NS, TN], F32R, tag="zt")
                nc.sync.dma_start(zt[:], zT[:, t * TN:(t + 1) * TN])
                a_ps = aps_pool.tile([128, 2, TN], F32, tag="aps")
                for j in range(2):
                    nc.tensor.matmul(a_ps[:, j], L1s[:, j * 128:(j + 1) * 128],
                                     zt[:], start=True, stop=True)
                S["a"] = a_ps
            elif part == 1:    # e1, lg1
                e = sbe.tile([128, 2, TN], BF16, tag="e")
                nc.scalar.activation(e[:], S["a"][:], AF.Exp, scale=-1.0)
                lg1 = sb.tile([128, 2, TN], BF16, tag="lg1")
                nc.scalar.activation(lg1[:], e[:], AF.Ln, bias=1.0)
                S["lg1"] = lg1
            elif part == 2:    # r1 (Act) + sg1 (Pool)
                r1 = sb.tile([128, 2, TN], BF16, tag="r1")
                nc.scalar.activation(r1[:], S["lg1"][:], AF.Exp, scale=-1.0)
                s1 = sb.tile([128, 2, TN], BF16, tag="s1")
                nc.gpsimd.tensor_scalar(s1[:], r1[:], -1.0 / ASC, 1.0 / ASC,
                                        OP.mult, OP.add)
                S["s1"] = s1
            elif part == 3:    # a2
                a_ps = aps_pool.tile([128, 2, TN], F32, tag="aps")
                for j in range(2):
                    for k in range(2):
                        nc.tensor.matmul(a_ps[:, j],
                                         L2s[:, k, j * 128:(j + 1) * 128],
                                         S["lg1"][:, k], start=(k == 0), stop=(k == 1))
                S["a"] = a_ps
            elif part == 4:    # e2, lg2, r2 (Act)
                e = sbe.tile([128, 2, TN], BF16, tag="e")
                nc.scalar.activation(e[:], S["a"][:], AF.Exp, scale=-1.0)
                lg2 = sb.tile([128, 2, TN], BF16, tag="lg2")
                nc.scalar.activation(lg2[:], e[:], AF.Ln, bias=1.0)
                S["lg2"] = lg2
                r2 = sb.tile([128, 2, TN], BF16, tag="r2")
                nc.scalar.activation(r2[:], lg2[:], AF.Exp, scale=-1.0)
                S["r2"] = r2
            elif part == 5:    # hm2 (DVE, on the spine)
                hm2 = sb.tile([128, 2, TN], BF16, tag="hm2")
                nc.vector.tensor_tensor(hm2[:], S["lg1"][:], S["lg2"][:], OP.add)
                S["hm2"] = hm2
            elif part == 6:    # sg2 (Pool, 1/8 for the fp8 W2 scale)
                s2 = sb.tile([128, 2, TN], BF16, tag="s2")
                nc.gpsimd.tensor_scalar(s2[:], S["r2"][:], -1.0, 1.0,
                                        OP.mult, OP.add)
                S["s2"] = s2
            elif part == 7:    # a3
                a_ps = aps_pool.tile([128, 2, TN], F32, tag="aps")
                for j in range(2):
                    for k in range(2):
                        nc.tensor.matmul(a_ps[:, j],
                                         L3s[:, k, j * 128:(j + 1) * 128],
                                         S["hm2"][:, k], start=(k == 0), stop=(k == 1))
                S["a"] = a_ps
            elif part == 8:    # e3, lg3, r3 (Act)
                e = sbe.tile([128, 2, TN], BF16, tag="e")
                nc.scalar.activation(e[:], S["a"][:], AF.Exp, scale=-1.0)
                lg3 = sb.tile([128, 2, TN], BF16, tag="lg3")
                nc.scalar.activation(lg3[:], e[:], AF.Ln, bias=1.0)
                S["lg3"] = lg3
                r3 = sb.tile([128, 2, TN], F8E4, tag="r3")
                nc.scalar.activation(r3[:], lg3[:], AF.Exp, scale=-1.0)
                S["r3"] = r3
            elif part == 9:    # hm3 (Pool, off-spine: feeds dt only)
                hm3 = sb.tile([128, 2, TN], BF16, tag="hm3")
                nc.gpsimd.tensor_tensor(hm3[:], S["hm2"][:], S["lg3"][:], OP.add)
                S["hm3"] = hm3
            elif part == 10:   # dt matmuls
                dt_ps = sup[s]["dt"]
                for g in range(4):
                    gi = w * 4 + g
                    for k in range(2):
                        nc.tensor.matmul(dt_ps[:, gi],
                                         S["hm3"][:, k, g * 128:(g + 1) * 128],
                                         L4s[:, k], start=(k == 0), stop=(k == 1))

        def bw_a(t, i):
            S = st[t]
            v_ps = vps_pool.tile([128, 2, TN], F32, tag="vps", name=f"vps{i}")
            S.setdefault("v", {})[i] = v_ps
            for j in range(2):
                nc.tensor.matmul(v_ps[:, j],
                                 M3s[:, :, i, j * 128:(j + 1) * 128],
                                 S["r3"][:], start=True, stop=False,
                                 perf_mode=mybir.MatmulPerfMode.DoubleRow)
                nc.tensor.matmul(v_ps[:, j], Cbs[:, i, j * 128:(j + 1) * 128],
                                 ones[:], start=False, stop=False)

        def bw_b(t, i):
            S = st[t]
            v2s2 = sbv.tile([128, 2, TN], BF16, tag="v2s2", name=f"v2s2_{i}")
            S.setdefault("v2", {})[i] = v2s2
            nc.vector.tensor_tensor(v2s2[:], S["v"][i][:], S["s2"][:], OP.mult)

        def bw_c(t, i):
            S = st[t]
            for j in range(2):
                for k in range(2):
                    nc.tensor.matmul(S["v"][i][:, j],
                                     W2s[:, k, j * 128:(j + 1) * 128],
                                     S["v2"][i][:, k], start=False, stop=(k == 1))

        def bw_d(t, i):
            S = st[t]
            v1s1 = sbv.tile([128, 2, TN], BF16, tag="v1s1", name=f"v1s1_{i}")
            S.setdefault("v1", {})[i] = v1s1
            nc.vector.tensor_tensor(v1s1[:], S["v"][i][:], S["s1"][:], OP.mult)

        def bw_e(t, i):
            S = st[t]
            w = t % SUP
            jt_ps = sup[t // SUP]["jt"]
            for g in range(4):
                gi = w * 4 + g
                for k in range(2):
                    nc.tensor.matmul(jt_ps[:, gi, i * NS:(i + 1) * NS],
                                     S["v1"][i][:, k, g * 128:(g + 1) * 128],
                                     W1s[:, k], start=(k == 0), stop=(k == 1))

        def emit_copies(t):
            # per-supertile: copy jt/dt accumulators out of PSUM (Act)
            w, s = t % SUP, t // SUP
            if w != SUP - 1:
                return
            sq = s % SOLVE_SUPS
            if sq == 0:
                grp["msb"] = mpool.tile([128, 10, SROWS], F32, tag="msb", name="msb")
                grp["vsb"] = mpool.tile([128, 4, SROWS], F32, tag="vsb", name="vsb")
                grp["csb"] = mpool.tile([128, SROWS, 4], F32, tag="csb", name="csb")
                grp["jtg"] = jpool.tile([128, SROWS, 4 * NS], BF16, tag="jtg", name="jtg")
                grp["dtg"] = jpool.tile([128, SROWS, NS], F32, tag="dtg", name="dtg")
                grp["nsup"] = 0
            rows = slice(sq * GRPS, (sq + 1) * GRPS)
            nc.scalar.copy(grp["jtg"][:, rows], sup[s]["jt"][:])
            nc.scalar.copy(grp["dtg"][:, rows], sup[s]["dt"][:])
            grp["nsup"] += 1
            sup[s]["jtg"] = grp["jtg"]
            sup[s]["dtg"] = grp["dtg"]
            sup[s]["msb"] = grp["msb"]
            sup[s]["vsb"] = grp["vsb"]
            sup[s]["csb"] = grp["csb"]
            sup[s]["sq"] = sq
            sup[s]["nsup"] = grp["nsup"]

        def emit_prods(t):
            # deferred Pool products for the supertile ending at tile t
            w, s = t % SUP, t // SUP
            if w != SUP - 1:
                return
            sq = sup[s]["sq"]
            rows = slice(sq * GRPS, (sq + 1) * GRPS)
            jts = sup[s]["jtg"][:, rows]
            dts = sup[s]["dtg"][:, rows]
            gprod = sb.tile([128, 10, GRPS, NS], BF16, tag="gprod")
            pbase = 0
            for a in range(4):
                nb = 4 - a
                in0 = jts[:, :, a * NS:(a + 1) * NS].unsqueeze(1) \
                    .to_broadcast((128, nb, GRPS, NS))
                in1 = jts[:, :, a * NS:].rearrange("p g (b n) -> p b g n", b=nb)
                nc.gpsimd.tensor_tensor(gprod[:, pbase:pbase + nb],
                                        in0, in1, OP.mult)
                pbase += nb
            vprod = sb.tile([128, 4, GRPS, NS], F32, tag="vprod")
            nc.gpsimd.tensor_tensor(
                vprod[:],
                jts[:].rearrange("p g (b n) -> p b g n", b=4),
                dts[:].unsqueeze(1).to_broadcast((128, 4, GRPS, NS)),
                OP.mult)
            sup[s]["gprod"] = gprod
            sup[s]["vprod"] = vprod

        def emit_reds(t):
            # deferred DVE reductions for the supertile ending at tile t
            w, s = t % SUP, t // SUP
            if w != SUP - 1:
                return
            sq = sup[s]["sq"]
            rows = slice(sq * GRPS, (sq + 1) * GRPS)
            nc.vector.tensor_reduce(
                sup[s]["msb"][:, :, rows].unsqueeze(3), sup[s]["gprod"][:],
                AX.X, OP.add)
            nc.vector.tensor_reduce(
                sup[s]["vsb"][:, :, rows].unsqueeze(3), sup[s]["vprod"][:],
                AX.X, OP.add)
            if sup[s]["sq"] == SOLVE_SUPS - 1 or t == ntiles - 1:
                enqueue_solve(s)

        def enqueue_solve(s):
            msb, vsb, csb = sup[s]["msb"], sup[s]["vsb"], sup[s]["csb"]
            jtg, dtg = sup[s]["jtg"], sup[s]["dtg"]
            nsup = sup[s]["nsup"]
            R = nsup * GRPS

            def m(i_):
                return msb[:, i_, :R].unsqueeze(2)

            def vv(i_):
                return vsb[:, i_, :R].unsqueeze(2)

            tt = {}

            def tmp(name):
                if name not in tt:
                    tt[name] = mpool.tile([128, SROWS, 1], F32, tag=f"t_{name}", name=f"t_{name}")
                return tt[name][:, :R]

            V = nc.gpsimd
            step_cost = 64 * 1.39 + 95
            k_now = cur_k[0]

            def q(*ops, lag=1):
                for op in ops:
                    pool_q.append((step_cost, k_now + lag, op))

            def mul(o, x, y):
                return lambda: V.tensor_tensor(o, x, y, OP.mult)

            def sub(o, x, y):
                return lambda: V.tensor_tensor(o, x, y, OP.subtract)

            def add(o, x, y):
                return lambda: V.tensor_tensor(o, x, y, OP.add)

            # lazily build AP closures (tiles exist already)
            m00, m01, m02, m03, m11, m12, m13, m22, m23, m33 = (m(i_) for i_ in range(10))
            x1, x2 = tmp("x1"), tmp("x2")
            detA, u10, u11 = tmp("detA"), tmp("u10"), tmp("u11")
            P00, P01, P10, P11 = tmp("P00"), tmp("P01"), tmp("P10"), tmp("P11")
            S00, S01, S11 = tmp("S00"), tmp("S01"), tmp("S11")
            w0, w1, detS = tmp("w0"), tmp("w1"), tmp("detS")
            cw2, cw3, q0, q1 = tmp("cw2"), tmp("cw3"), tmp("q0"), tmp("q1")
            dAS, rAS, rS = tmp("dAS"), tmp("rAS"), tmp("rS")
            x3, x4 = tmp("x3"), tmp("x4")

            q(mul(x1, m00, m11), mul(x2, m01, m01), sub(detA, x1, x2))
            q(mul(x1, m11, vv(0)), mul(x2, m01, vv(1)), sub(u10, x1, x2))
            q(mul(x1, m00, vv(1)), mul(x2, m01, vv(0)), sub(u11, x1, x2))
            q(mul(x1, m11, m02), mul(x2, m01, m12), sub(P00, x1, x2))
            q(mul(x1, m11, m03), mul(x2, m01, m13), sub(P01, x1, x2))
            q(mul(x1, m00, m12), mul(x2, m01, m02), sub(P10, x1, x2))
            q(mul(x1, m00, m13), mul(x2, m01, m03), sub(P11, x1, x2))
            q(mul(x1, m02, P00), mul(x2, m12, P10), add(x1, x1, x2))
            q(mul(S00, detA, m22), sub(S00, S00, x1))
            q(mul(x3, m02, P01), mul(x4, m12, P11), add(x3, x3, x4))
            q(mul(S01, detA, m23), sub(S01, S01, x3))
            q(mul(x1, m03, P01), mul(x2, m13, P11), add(x1, x1, x2))
            q(mul(S11, detA, m33), sub(S11, S11, x1))
            q(mul(x3, m02, u10), mul(x4, m12, u11), add(x3, x3, x4))
            q(mul(w0, detA, vv(2)), sub(w0, w0, x3))
            q(mul(x1, m03, u10), mul(x2, m13, u11), add(x1, x1, x2))
            q(mul(w1, detA, vv(3)), sub(w1, w1, x1))
            q(mul(x3, S00, S11), mul(x4, S01, S01), sub(detS, x3, x4))
            q(mul(x1, S11, w0), mul(x2, S01, w1), sub(cw2, x1, x2))
            q(mul(x3, S00, w1), mul(x4, S01, w0), sub(cw3, x3, x4))
            q(mul(x1, P00, cw2), mul(x2, P01, cw3), add(x1, x1, x2))
            q(mul(q0, u10, detS), sub(q0, q0, x1))
            q(mul(x3, P10, cw2), mul(x4, P11, cw3), add(x3, x3, x4))
            q(mul(q1, u11, detS), sub(q1, q1, x3))
            q(mul(dAS, detA, detS))
            q(lambda: nc.vector.reciprocal(rAS, dAS), lag=5)
            q(lambda: nc.vector.reciprocal(rS, detS), lag=5)
            q(mul(csb[:, :R, 0:1], q0, rAS), mul(csb[:, :R, 1:2], q1, rAS), lag=5)
            q(mul(csb[:, :R, 2:3], cw2, rS), mul(csb[:, :R, 3:4], cw3, rS), lag=5)

            # combine + output DMA, sliced into quarter-row chunks so the
            # Pool stream never lumps and delays spine-critical sg ops
            s_base = s - (nsup - 1)
            R2 = nsup * GRPS
            acc = sb.tile([128, SROWS, NS], F32, tag="acc")
            ctmp = sb.tile([128, SROWS, NS], F32, tag="ctmp")
            RQ = R2 // 4
            slice_cost = RQ * NS * 1.99 + 95

            def comb(a, r0, r1):
                def fn():
                    cb = csb[:, r0:r1, a:a + 1].to_broadcast((128, r1 - r0, NS))
                    V.tensor_tensor(ctmp[:, r0:r1], cb,
                                    jtg[:, r0:r1, a * NS:(a + 1) * NS], OP.mult)
                    if a == 0:
                        V.tensor_tensor(acc[:, r0:r1], dtg[:, r0:r1],
                                        ctmp[:, r0:r1], OP.subtract)
                    else:
                        V.tensor_tensor(acc[:, r0:r1], acc[:, r0:r1],
                                        ctmp[:, r0:r1], OP.subtract)
                return fn

            for a in range(4):
                for rq in range(4):
                    pool_q.append((2 * slice_cost, k_now + 5 + a,
                                   comb(a, rq * RQ, (rq + 1) * RQ)))

            def out_dma():
                eb = s_base * SUP * TN
                nc.sync.dma_start(
                    out_d[eb:eb + R2 * 128, :].rearrange("(g p) m -> p g m", p=128),
                    acc[:, :R2])

            pool_q.append((0.0, k_now + 10, out_dma))

        # ---------------- the scheduled iterations
        # Tail ops (stage_e, dt, copies) are deferred one iteration so each
        # iteration's PE stream STARTS with the next tile's forward spine.
        for k in range(ntiles + 4):
            tb = k - 1           # backward tile (stages a-d)
            tf = k               # forward tile
            te = k - 2           # deferred stage_e / copies tile
            td = k - 1           # deferred dt tile
            tp = k - 3           # deferred postproc tile (prods/reduces)
            fw = tf < ntiles
            bw = 0 <= tb < ntiles
            ee = 0 <= te < ntiles
            dd = 0 <= td < ntiles
            pp = 0 <= tp < ntiles

            if fw: emit_fwd(tf, 0)                      # PE: a1 (+dma)
            if bw: bw_a(tb, 0); bw_a(tb, 1)             # PE: A0 A1
            if fw: emit_fwd(tf, 1)                      # Act: e1 lg1
            if bw: bw_a(tb, 2)                          # PE: A2
            if ee:
                bw_e(te, 0); bw_e(te, 1)                # PE (deferred)
                bw_e(te, 2); bw_e(te, 3)
            if fw: emit_fwd(tf, 2)                      # Act: r1 ; Pool: sg1
            if ee: emit_copies(te)                      # Act (fills a2-wait gap)
            if pp: emit_prods(tp)                       # Pool: gram/vprod
            if bw:
                bw_b(tb, 0); bw_b(tb, 1)                # DVE
                bw_b(tb, 2)
            if fw: emit_fwd(tf, 3)                      # PE: a2
            if bw: bw_c(tb, 0); bw_c(tb, 1)             # PE
            if dd: emit_fwd(td, 10)                     # PE: dt (deferred)
            if bw: bw_d(tb, 0)                          # DVE
            if fw: emit_fwd(tf, 4)                      # Act: e2 lg2 r2
            if fw: emit_fwd(tf, 6)                      # Pool: sg2
            if fw: emit_fwd(tf, 5)                      # DVE: hm2 (after D0)
            if bw: bw_d(tb, 1)                          # DVE
            if fw: emit_fwd(tf, 7)                      # PE: a3 (before A3!)
            if bw: bw_a(tb, 3)                          # PE: A3 (v3 <- D0)
            if bw: bw_b(tb, 3)                          # DVE
            if bw: bw_c(tb, 2); bw_c(tb, 3)             # PE
            if bw: bw_d(tb, 2); bw_d(tb, 3)             # DVE
            if pp: emit_reds(tp)                        # DVE (+ enqueue solve)
            if fw: emit_fwd(tf, 8)                      # Act: e3 lg3 r3
            if fw: emit_fwd(tf, 9)                      # Pool: hm3
            cur_k[0] = k
            drain_pool_q(2400)
            if te - 1 >= 0:
                st.pop(te - 1, None)

        while pool_q:
            cost, mink, fn = pool_q.popleft()
            fn()

    _split_multi_waits(nc)
    return nc


# ---------------------------------------------------------------- entry point
def kernel(zstates, W1, W2, W3, W4):
    from concourse.bass_utils import run_bass_kernel_spmd

    key = "full"
    if key not in _cache:
        _cache[key] = _build()
    nc = _cache[key]

    wm = _prep_weights(W1, W2, W3, W4)
    z = np.asarray(zstates, np.float32).reshape(NCORES, BP, NS)
    in_maps = [
        {**wm, "zT": np.ascontiguousarray(z[c].T)} for c in range(NCORES)
    ]
    res = run_bass_kernel_spmd(nc, in_maps, core_ids=list(range(NCORES)))
    return np.concatenate([res.results[c]["out"] for c in range(NCORES)], axis=0)



# revision 4
# speedup vs baseline: 1.1816x; 1.1816x over previous
"""nn_CoMet Trainium2 kernel (v2).

Math (per batch element s in R^16):
  MLP: h1 = logsig(s@W1.T); h2 = h1 + logsig(h1@W2.T); h3 = h2 + logsig(h2@W3.T)
       nnout = h3@W4.T ; d = nnout[:16]; com-rows C = W4[16:20]
  J = d com/d s  (reverse mode through the MLP, 4 rows)
  out = d - J^T (J J^T)^{-1} J d      (== the QR-based projection in the reference)

Kernel design (per core, batch 32768, hidden-major layout [hidden on partitions,
elements on free axis], tiles of 512 elements):
  - forward: activations via the natural_log_exp table only: e=Exp(-a),
    lg=Ln(e+1) (= -logsig(a) = hm increment), r=Exp(-lg) (= sigmoid(a));
    sg = 1-r on DVE tensor_scalar (4x mode).  a3 takes lg1 and lg2 as two
    separate moving passes so the spine never waits on the DVE hm2 add.
  - backward per com-row i: stage_a computes v = -C_i diag(r3) W3 into PSUM
    via fp8 DoubleRow matmuls (stationary -ASC*diag(C_i)W3, moving r3).  The
    residual bias B_i = ASC*(C_i + C_i@W3) is NOT materialized in PSUM;
    instead both PSUM reads apply it inline with scalar_tensor_tensor:
      v2s2 = (v + B_i) * sg2   (j-half 0 on DVE, half 1 on Pool, in parallel)
      v1s1 = (v + B_i) * sg1   (same split)
    bw_c accumulates W2^T v2s2 on top of the same PSUM.  The 1/ASC scale is
    folded into the bw_e stationary W1/ASC.
  - J and d produced transposed (element-major); Gram products on DVE (2x),
    Jd products on Pool; 4x4 normal-equation solve batched over 8 supertiles
    on Pool; combine on DVE.
"""

import numpy as np
import ml_dtypes

import concourse.bass as bass
import concourse.mybir as mybir
import concourse.tile as tile

NCORES = 8
B = 262144
BP = B // NCORES          # 32768 per core
NS = 16
NH = 256
NCOM = 4
TN = 512                  # elements per pipeline tile
NTILES = BP // TN         # 64
SUP = 2                   # tiles per supertile (postproc unit) -> 1024 elems
GRPS = SUP * TN // 128    # 8 groups of 128 elems per supertile
SOLVE_SUPS = 8            # supertiles per batched solve -> 8192 elems
SROWS = SOLVE_SUPS * GRPS # 64
ASC = 128.0               # backward fp8 scale (M3) ; removed via W1b

F32 = mybir.dt.float32
F32R = mybir.dt.float32r
BF16 = mybir.dt.bfloat16
F8E4 = mybir.dt.float8e4
AF = mybir.ActivationFunctionType
OP = mybir.AluOpType
AX = mybir.AxisListType

_cache = {}


# ---------------------------------------------------------------- drain patch
def _patch_tile_drain():
    """walrus in this container rejects >1 sem wait on the TileContext final
    drain ("Too many sync wait commands"); split the waits across several
    drain instructions (1 wait each)."""
    if getattr(tile.TileContext, "_comet_patched", False):
        return
    from concourse.vector_clock import ScopedClock

    def _drain_and_barrier(self, tick_clock, wait_clock):
        nc = self.nc
        drain_inst = nc.sync.drain()
        wait_clock.add_sem_waits(
            drain_inst.ins, ScopedClock({None: tick_clock.global_clock})
        )
        si = drain_inst.ins.sync_info
        waits = list(si.on_wait) if si is not None and si.on_wait else []
        if len(waits) > 1:
            si.on_wait = waits[:1]
            for w in waits[1:]:
                extra = nc.sync.drain()
                esi = extra.ins.sync_info
                if esi is None:
                    import bass_rust
                    extra.ins.sync_info = bass_rust.SyncInfo(
                        on_wait=[w], on_update=[]
                    )
                else:
                    esi.on_wait = list(esi.on_wait or []) + [w]
        nc.all_engine_barrier()
        assert self.sems is not None
        popped = nc._tile_sem_poison_stack.pop()
        assert popped is self._sem_poison
        nc.clear_and_free_semaphores(list(self.sems.allocated().values()))
        nc.all_engine_barrier()

    tile.TileContext._drain_and_barrier = _drain_and_barrier
    tile.TileContext._comet_patched = True


def _split_multi_waits(nc):
    """This container's walrus rejects instructions with more than one sync
    wait command.  Hoist extra waits onto injected same-engine NOPs placed
    immediately before the offending instruction."""
    import bass_rust

    for f in nc.m.functions:
        for b in f.blocks:
            insts = list(b.instructions)
            out, dirty = [], False
            for inst in insts:
                si = inst.sync_info
                waits = list(si.on_wait) if si is not None and si.on_wait else []
                if len(waits) > 1:
                    dirty = True
                    for k, wx in enumerate(waits[:-1]):
                        nop = mybir.InstNoOp(name=f"{inst.name}-ws{k}")
                        nop.engine = inst.engine
                        nop.sync_info = bass_rust.SyncInfo(
                            on_wait=[wx], on_update=[])
                        out.append(nop)
                    si.on_wait = waits[-1:]
                out.append(inst)
            if dirty:
                b.instructions = out


# ---------------------------------------------------------------- host prep
def _prep_weights(W1, W2, W3, W4):
    W1 = np.asarray(W1, np.float32)
    W2 = np.asarray(W2, np.float32)
    W3 = np.asarray(W3, np.float32)
    W4 = np.asarray(W4, np.float32)
    W4d, C = W4[:NS], W4[NS:]
    bf = ml_dtypes.bfloat16

    def khalf(a, m):   # [256, m] -> [128, 2, m]
        return np.ascontiguousarray(a.reshape(2, 128, m).transpose(1, 0, 2))

    f8 = ml_dtypes.float8_e4m3
    L1 = np.ascontiguousarray(W1.T)                       # [16,256] lhsT for a1
    L2 = khalf(-W2.T, NH).astype(bf)                      # [128,2,256]
    L3 = khalf(-W3.T, NH).astype(bf)
    L4 = khalf(-W4d.T, NS).astype(bf)                     # [128,2,16]
    # stage_a stationary: -diag(C_i)W3 (moving operand is r3 = sigmoid(a3)),
    # scaled by ASC and quantized to fp8 e4m3 for the DoubleRow matmul; the
    # scale is removed via the 1/ASC factor folded into W1b.
    M3 = np.stack([-C[i][:, None] * W3 for i in range(4)])  # [4,256,256]
    M3 = np.ascontiguousarray(
        (ASC * M3).reshape(4, 2, 128, NH).transpose(2, 1, 0, 3)  # [128,2,4,256]
    ).astype(f8)
    W2b = khalf(W2, NH).astype(bf)                        # [128,2,256] lhsT for U
    W1b = khalf(W1 / ASC, NS).astype(bf)                  # [128,2,16]  rhs for Jt
    # bias B_i = ASC*(C_i + C_i@W3): folds the residual C_i and the (1-r3)
    # complement of the layer-3 backward; applied per-partition by the
    # scalar_tensor_tensor PSUM reads (stages b and d).
    Bm = ASC * (C + C @ W3)                               # [4, 256]
    Bs = np.ascontiguousarray(
        Bm.reshape(4, 2, 128).transpose(2, 1, 0)          # [128, 2, 4]
    ).astype(np.float32)
    return {
        "L1": L1, "L2": L2, "L3": L3, "L4": L4,
        "M3": M3, "W2b": W2b, "W1b": W1b, "Bs": Bs,
    }


# ---------------------------------------------------------------- build
def _build(ntiles=NTILES, a_bufs=1, v_bufs=2):
    _patch_tile_drain()
    nc = bass.Bass()

    zT = nc.dram_tensor("zT", [NS, BP], F32R, kind="ExternalInput")
    dL1 = nc.dram_tensor("L1", [NS, NH], F32R, kind="ExternalInput")
    dL2 = nc.dram_tensor("L2", [128, 2, NH], BF16, kind="ExternalInput")
    dL3 = nc.dram_tensor("L3", [128, 2, NH], BF16, kind="ExternalInput")
    dL4 = nc.dram_tensor("L4", [128, 2, NS], BF16, kind="ExternalInput")
    dM3 = nc.dram_tensor("M3", [128, 2, 4, NH], F8E4, kind="ExternalInput")
    dW2b = nc.dram_tensor("W2b", [128, 2, NH], BF16, kind="ExternalInput")
    dW1b = nc.dram_tensor("W1b", [128, 2, NS], BF16, kind="ExternalInput")
    dBs = nc.dram_tensor("Bs", [128, 2, 4], F32, kind="ExternalInput")
    out_d = nc.dram_tensor("out", [BP, NS], F32, kind="ExternalOutput")

    from contextlib import ExitStack
    with tile.TileContext(nc) as tc, ExitStack() as ctx:
        wpool = ctx.enter_context(tc.tile_pool(name="w", bufs=1))
        sb = ctx.enter_context(tc.tile_pool(name="sb", bufs=3))
        sbe = ctx.enter_context(tc.tile_pool(name="sbe", bufs=6))
        sbv = ctx.enter_context(tc.tile_pool(name="sbv", bufs=6))
        sbz = ctx.enter_context(tc.tile_pool(name="sbz", bufs=6))
        jpool = ctx.enter_context(tc.tile_pool(name="jp", bufs=2))
        mpool = ctx.enter_context(tc.tile_pool(name="mp", bufs=2))
        aps_pool = ctx.enter_context(tc.tile_pool(name="aps", bufs=a_bufs, space="PSUM"))
        vps_pool = ctx.enter_context(tc.tile_pool(name="vps", bufs=v_bufs, space="PSUM"))
        tps_pool = ctx.enter_context(tc.tile_pool(name="tps", bufs=1, space="PSUM"))

        # ---- load constants
        L1s = wpool.tile([NS, NH], F32R)
        nc.sync.dma_start(L1s[:], dL1[:])
        L2s = wpool.tile([128, 2, NH], BF16)
        nc.sync.dma_start(L2s[:], dL2[:])
        L3s = wpool.tile([128, 2, NH], BF16)
        nc.sync.dma_start(L3s[:], dL3[:])
        L4s = wpool.tile([128, 2, NS], BF16)
        nc.sync.dma_start(L4s[:], dL4[:])
        M3s = wpool.tile([128, 2, 4, NH], F8E4)
        nc.sync.dma_start(M3s[:], dM3[:])
        W2s = wpool.tile([128, 2, NH], BF16)
        nc.sync.dma_start(W2s[:], dW2b[:])
        W1s = wpool.tile([128, 2, NS], BF16)
        nc.sync.dma_start(W1s[:], dW1b[:])
        Bsb = wpool.tile([128, 2, 4], F32)
        nc.sync.dma_start(Bsb[:], dBs[:])

        # ---- hand-scheduled software pipeline.
        from collections import deque

        st = {}        # per-tile forward state
        sup = {}       # per-supertile PSUM accumulators
        grp = {"nsup": 0}
        pool_q = deque()   # (cost_ns, min_iter, closure) deferred solve/combine
        cur_k = [0]

        def drain_pool_q(budget):
            while pool_q and budget > 0:
                cost, mink, fn = pool_q[0]
                if mink > cur_k[0]:
                    break
                pool_q.popleft()
                fn()
                budget -= cost

        def emit_fwd(t, part):
            S = st.setdefault(t, {})
            w, s = t % SUP, t // SUP
            if part == 0:      # dma + a1
                if w == 0:
                    sup[s] = {
                        "jt": tps_pool.tile([128, GRPS, 4 * NS], F32, tag="jtps", name="jt_ps"),
                        "dt": tps_pool.tile([128, GRPS, NS], F32, tag="dtps", name="dt_ps"),
                    }
                zt = sbz.tile([NS, TN], F32R, tag="zt")
                nc.sync.dma_start(zt[:], zT[:, t * TN:(t + 1) * TN])
                a_ps = aps_pool.tile([128, 2, TN], F32, tag="aps")
                for j in range(2):
                    nc.tensor.matmul(a_ps[:, j], L1s[:, j * 128:(j + 1) * 128],
                                     zt[:], start=True, stop=True)
                S["a"] = a_ps
            elif part == 1:    # e1, lg1  (Act)
                e = sbe.tile([128, 2, TN], BF16, tag="e")
                nc.scalar.activation(e[:], S["a"][:], AF.Exp, scale=-1.0)
                lg1 = sb.tile([128, 2, TN], BF16, tag="lg1")
                nc.scalar.activation(lg1[:], e[:], AF.Ln, bias=1.0)
                S["lg1"] = lg1
            elif part == 2:    # r1 (Act) + sg1 (DVE 4x)
                r1 = sb.tile([128, 2, TN], BF16, tag="r1")
                nc.scalar.activation(r1[:], S["lg1"][:], AF.Exp, scale=-1.0)
                s1 = sb.tile([128, 2, TN], BF16, tag="s1")
                nc.vector.tensor_scalar(s1[:], r1[:], -1.0, 1.0,
                                        OP.mult, OP.add)
                S["s1"] = s1
            elif part == 3:    # a2
                a_ps = aps_pool.tile([128, 2, TN], F32, tag="aps")
                for j in range(2):
                    for k in range(2):
                        nc.tensor.matmul(a_ps[:, j],
                                         L2s[:, k, j * 128:(j + 1) * 128],
                                         S["lg1"][:, k], start=(k == 0), stop=(k == 1))
                S["a"] = a_ps
            elif part == 4:    # e2, lg2, r2 (Act)
                e = sbe.tile([128, 2, TN], BF16, tag="e")
                nc.scalar.activation(e[:], S["a"][:], AF.Exp, scale=-1.0)
                lg2 = sb.tile([128, 2, TN], BF16, tag="lg2")
                nc.scalar.activation(lg2[:], e[:], AF.Ln, bias=1.0)
                S["lg2"] = lg2
                r2 = sb.tile([128, 2, TN], BF16, tag="r2")
                nc.scalar.activation(r2[:], lg2[:], AF.Exp, scale=-1.0)
                S["r2"] = r2
            elif part == 5:    # hm2 (DVE 2x, off-spine: feeds hm3/dt only)
                hm2 = sb.tile([128, 2, TN], BF16, tag="hm2")
                nc.vector.tensor_tensor(hm2[:], S["lg1"][:], S["lg2"][:], OP.add)
                S["hm2"] = hm2
            elif part == 6:    # sg2 (DVE 4x)
                s2 = sb.tile([128, 2, TN], BF16, tag="s2")
                nc.vector.tensor_scalar(s2[:], S["r2"][:], -1.0, 1.0,
                                        OP.mult, OP.add)
                S["s2"] = s2
            elif part == 71:   # a3 first half: W3 @ lg1 (spine-free)
                a_ps = aps_pool.tile([128, 2, TN], F32, tag="aps")
                for j in range(2):
                    for k in range(2):
                        nc.tensor.matmul(a_ps[:, j],
                                         L3s[:, k, j * 128:(j + 1) * 128],
                                         S["lg1"][:, k], start=(k == 0), stop=False)
                S["a"] = a_ps
            elif part == 72:   # a3 second half: W3 @ lg2 (waits only on lg2)
                a_ps = S["a"]
                for j in range(2):
                    for k in range(2):
                        nc.tensor.matmul(a_ps[:, j],
                                         L3s[:, k, j * 128:(j + 1) * 128],
                                         S["lg2"][:, k], start=False, stop=(k == 1))
            elif part == 8:    # e3, lg3, r3 (Act)
                e = sbe.tile([128, 2, TN], BF16, tag="e")
                nc.scalar.activation(e[:], S["a"][:], AF.Exp, scale=-1.0)
                lg3 = sb.tile([128, 2, TN], BF16, tag="lg3")
                nc.scalar.activation(lg3[:], e[:], AF.Ln, bias=1.0)
                S["lg3"] = lg3
                r3 = sb.tile([128, 2, TN], F8E4, tag="r3")
                nc.scalar.activation(r3[:], lg3[:], AF.Exp, scale=-1.0)
                S["r3"] = r3
            elif part == 9:    # hm3 (DVE 2x, off-spine: feeds dt only)
                hm3 = sb.tile([128, 2, TN], BF16, tag="hm3")
                nc.vector.tensor_tensor(hm3[:], S["hm2"][:], S["lg3"][:], OP.add)
                S["hm3"] = hm3
            elif part == 10:   # dt matmuls
                dt_ps = sup[s]["dt"]
                for g in range(4):
                    gi = w * 4 + g
                    for k in range(2):
                        nc.tensor.matmul(dt_ps[:, gi],
                                         S["hm3"][:, k, g * 128:(g + 1) * 128],
                                         L4s[:, k], start=(k == 0), stop=(k == 1))

        def bw_a(t, i):
            S = st[t]
            v_ps = vps_pool.tile([128, 2, TN], F32, tag="vps", name=f"vps{i}")
            S.setdefault("v", {})[i] = v_ps
            for j in range(2):
                nc.tensor.matmul(v_ps[:, j],
                                 M3s[:, :, i, j * 128:(j + 1) * 128],
                                 S["r3"][:], start=True, stop=False,
                                 perf_mode=mybir.MatmulPerfMode.DoubleRow)

        def bw_b(t, i):
            # v2s2 = (v + B_i) * sg2, split per j-half: DVE and Pool in parallel
            S = st[t]
            v2s2 = sbv.tile([128, 2, TN], BF16, tag="v2s2", name=f"v2s2_{i}")
            S.setdefault("v2", {})[i] = v2s2
            nc.vector.scalar_tensor_tensor(
                v2s2[:, 0], S["v"][i][:, 0], Bsb[:, 0, i:i + 1], S["s2"][:, 0],
                OP.add, OP.mult)
            nc.gpsimd.scalar_tensor_tensor(
                v2s2[:, 1], S["v"][i][:, 1], Bsb[:, 1, i:i + 1], S["s2"][:, 1],
                OP.add, OP.mult)

        def bw_c(t, i):
            S = st[t]
            for j in range(2):
                for k in range(2):
                    nc.tensor.matmul(S["v"][i][:, j],
                                     W2s[:, k, j * 128:(j + 1) * 128],
                                     S["v2"][i][:, k], start=False, stop=(k == 1))

        def bw_d(t, i):
            # v1s1 = (v + B_i) * sg1, split per j-half: DVE and Pool in parallel
            S = st[t]
            v1s1 = sbv.tile([128, 2, TN], BF16, tag="v1s1", name=f"v1s1_{i}")
            S.setdefault("v1", {})[i] = v1s1
            nc.vector.scalar_tensor_tensor(
                v1s1[:, 0], S["v"][i][:, 0], Bsb[:, 0, i:i + 1], S["s1"][:, 0],
                OP.add, OP.mult)
            nc.gpsimd.scalar_tensor_tensor(
                v1s1[:, 1], S["v"][i][:, 1], Bsb[:, 1, i:i + 1], S["s1"][:, 1],
                OP.add, OP.mult)

        def bw_e(t, i):
            S = st[t]
            w = t % SUP
            jt_ps = sup[t // SUP]["jt"]
            for g in range(4):
                gi = w * 4 + g
                for k in range(2):
                    nc.tensor.matmul(jt_ps[:, gi, i * NS:(i + 1) * NS],
                                     S["v1"][i][:, k, g * 128:(g + 1) * 128],
                                     W1s[:, k], start=(k == 0), stop=(k == 1))

        def emit_copies(t):
            # per-supertile: copy jt/dt accumulators out of PSUM (Pool)
            w, s = t % SUP, t // SUP
            if w != SUP - 1:
                return
            sq = s % SOLVE_SUPS
            if sq == 0:
                grp["msb"] = mpool.tile([128, 10, SROWS], F32, tag="msb", name="msb")
                grp["vsb"] = mpool.tile([128, 4, SROWS], F32, tag="vsb", name="vsb")
                grp["csb"] = mpool.tile([128, SROWS, 4], F32, tag="csb", name="csb")
                grp["jtg"] = jpool.tile([128, SROWS, 4 * NS], BF16, tag="jtg", name="jtg")
                grp["dtg"] = jpool.tile([128, SROWS, NS], F32, tag="dtg", name="dtg")
                grp["nsup"] = 0
            rows = slice(sq * GRPS, (sq + 1) * GRPS)
            nc.gpsimd.tensor_copy(grp["jtg"][:, rows], sup[s]["jt"][:])
            nc.gpsimd.tensor_copy(grp["dtg"][:, rows], sup[s]["dt"][:])
            grp["nsup"] += 1
            sup[s]["jtg"] = grp["jtg"]
            sup[s]["dtg"] = grp["dtg"]
            sup[s]["msb"] = grp["msb"]
            sup[s]["vsb"] = grp["vsb"]
            sup[s]["csb"] = grp["csb"]
            sup[s]["sq"] = sq
            sup[s]["nsup"] = grp["nsup"]

        def emit_prods(t):
            # deferred products for the supertile ending at tile t
            w, s = t % SUP, t // SUP
            if w != SUP - 1:
                return
            sq = sup[s]["sq"]
            rows = slice(sq * GRPS, (sq + 1) * GRPS)
            jts = sup[s]["jtg"][:, rows]
            dts = sup[s]["dtg"][:, rows]
            gprod = sb.tile([128, 10, GRPS, NS], BF16, tag="gprod")
            pbase = 0
            for a in range(4):
                nb = 4 - a
                in0 = jts[:, :, a * NS:(a + 1) * NS].unsqueeze(1) \
                    .to_broadcast((128, nb, GRPS, NS))
                in1 = jts[:, :, a * NS:].rearrange("p g (b n) -> p b g n", b=nb)
                nc.vector.tensor_tensor(gprod[:, pbase:pbase + nb],
                                        in0, in1, OP.mult)
                pbase += nb
            vprod = sb.tile([128, 4, GRPS, NS], F32, tag="vprod")
            nc.gpsimd.tensor_tensor(
                vprod[:],
                jts[:].rearrange("p g (b n) -> p b g n", b=4),
                dts[:].unsqueeze(1).to_broadcast((128, 4, GRPS, NS)),
                OP.mult)
            sup[s]["gprod"] = gprod
            sup[s]["vprod"] = vprod

        def emit_reds(t):
            # deferred reductions for the supertile ending at tile t
            w, s = t % SUP, t // SUP
            if w != SUP - 1:
                return
            sq = sup[s]["sq"]
            rows = slice(sq * GRPS, (sq + 1) * GRPS)
            nc.vector.tensor_reduce(
                sup[s]["msb"][:, :, rows].unsqueeze(3), sup[s]["gprod"][:],
                AX.X, OP.add)
            nc.vector.tensor_reduce(
                sup[s]["vsb"][:, :, rows].unsqueeze(3), sup[s]["vprod"][:],
                AX.X, OP.add)
            if sup[s]["sq"] == SOLVE_SUPS - 1 or t == ntiles - 1:
                enqueue_solve(s)

        def enqueue_solve(s):
            msb, vsb, csb = sup[s]["msb"], sup[s]["vsb"], sup[s]["csb"]
            jtg, dtg = sup[s]["jtg"], sup[s]["dtg"]
            nsup = sup[s]["nsup"]
            R = nsup * GRPS

            def m(i_):
                return msb[:, i_, :R].unsqueeze(2)

            def vv(i_):
                return vsb[:, i_, :R].unsqueeze(2)

            tt = {}

            def tmp(name):
                if name not in tt:
                    tt[name] = mpool.tile([128, SROWS, 1], F32, tag=f"t_{name}", name=f"t_{name}")
                return tt[name][:, :R]

            V = nc.gpsimd
            step_cost = 64 * 1.39 + 95
            k_now = cur_k[0]

            def q(*ops, lag=1):
                for op in ops:
                    pool_q.append((step_cost, k_now + lag, op))

            def mul(o, x, y):
                return lambda: V.tensor_tensor(o, x, y, OP.mult)

            def sub(o, x, y):
                return lambda: V.tensor_tensor(o, x, y, OP.subtract)

            def add(o, x, y):
                return lambda: V.tensor_tensor(o, x, y, OP.add)

            # lazily build AP closures (tiles exist already)
            m00, m01, m02, m03, m11, m12, m13, m22, m23, m33 = (m(i_) for i_ in range(10))
            x1, x2 = tmp("x1"), tmp("x2")
            detA, u10, u11 = tmp("detA"), tmp("u10"), tmp("u11")
            P00, P01, P10, P11 = tmp("P00"), tmp("P01"), tmp("P10"), tmp("P11")
            S00, S01, S11 = tmp("S00"), tmp("S01"), tmp("S11")
            w0, w1, detS = tmp("w0"), tmp("w1"), tmp("detS")
            cw2, cw3, q0, q1 = tmp("cw2"), tmp("cw3"), tmp("q0"), tmp("q1")
            dAS, rAS, rS = tmp("dAS"), tmp("rAS"), tmp("rS")
            x3, x4 = tmp("x3"), tmp("x4")

            q(mul(x1, m00, m11), mul(x2, m01, m01), sub(detA, x1, x2))
            q(mul(x1, m11, vv(0)), mul(x2, m01, vv(1)), sub(u10, x1, x2))
            q(mul(x1, m00, vv(1)), mul(x2, m01, vv(0)), sub(u11, x1, x2))
            q(mul(x1, m11, m02), mul(x2, m01, m12), sub(P00, x1, x2))
            q(mul(x1, m11, m03), mul(x2, m01, m13), sub(P01, x1, x2))
            q(mul(x1, m00, m12), mul(x2, m01, m02), sub(P10, x1, x2))
            q(mul(x1, m00, m13), mul(x2, m01, m03), sub(P11, x1, x2))
            q(mul(x1, m02, P00), mul(x2, m12, P10), add(x1, x1, x2))
            q(mul(S00, detA, m22), sub(S00, S00, x1))
            q(mul(x3, m02, P01), mul(x4, m12, P11), add(x3, x3, x4))
            q(mul(S01, detA, m23), sub(S01, S01, x3))
            q(mul(x1, m03, P01), mul(x2, m13, P11), add(x1, x1, x2))
            q(mul(S11, detA, m33), sub(S11, S11, x1))
            q(mul(x3, m02, u10), mul(x4, m12, u11), add(x3, x3, x4))
            q(mul(w0, detA, vv(2)), sub(w0, w0, x3))
            q(mul(x1, m03, u10), mul(x2, m13, u11), add(x1, x1, x2))
            q(mul(w1, detA, vv(3)), sub(w1, w1, x1))
            q(mul(x3, S00, S11), mul(x4, S01, S01), sub(detS, x3, x4))
            q(mul(x1, S11, w0), mul(x2, S01, w1), sub(cw2, x1, x2))
            q(mul(x3, S00, w1), mul(x4, S01, w0), sub(cw3, x3, x4))
            q(mul(x1, P00, cw2), mul(x2, P01, cw3), add(x1, x1, x2))
            q(mul(q0, u10, detS), sub(q0, q0, x1))
            q(mul(x3, P10, cw2), mul(x4, P11, cw3), add(x3, x3, x4))
            q(mul(q1, u11, detS), sub(q1, q1, x3))
            q(mul(dAS, detA, detS))
            q(lambda: nc.vector.reciprocal(rAS, dAS), lag=5)
            q(lambda: nc.vector.reciprocal(rS, detS), lag=5)
            q(mul(csb[:, :R, 0:1], q0, rAS), mul(csb[:, :R, 1:2], q1, rAS), lag=5)
            q(mul(csb[:, :R, 2:3], cw2, rS), mul(csb[:, :R, 3:4], cw3, rS), lag=5)

            # combine + output DMA on DVE, sliced into quarter-row chunks
            s_base = s - (nsup - 1)
            R2 = nsup * GRPS
            acc = sb.tile([128, SROWS, NS], F32, tag="acc")
            ctmp = sb.tile([128, SROWS, NS], F32, tag="ctmp")
            RQ = R2 // 4
            slice_cost = RQ * NS * 1.04 + 60

            def comb(a, r0, r1):
                def fn():
                    cb = csb[:, r0:r1, a:a + 1].to_broadcast((128, r1 - r0, NS))
                    nc.vector.tensor_tensor(ctmp[:, r0:r1], cb,
                                            jtg[:, r0:r1, a * NS:(a + 1) * NS], OP.mult)
                    if a == 0:
                        nc.vector.tensor_tensor(acc[:, r0:r1], dtg[:, r0:r1],
                                                ctmp[:, r0:r1], OP.subtract)
                    else:
                        nc.vector.tensor_tensor(acc[:, r0:r1], acc[:, r0:r1],
                                                ctmp[:, r0:r1], OP.subtract)
                return fn

            for a in range(4):
                for rq in range(4):
                    pool_q.append((2 * slice_cost, k_now + 5 + a,
                                   comb(a, rq * RQ, (rq + 1) * RQ)))

            def out_dma():
                eb = s_base * SUP * TN
                nc.sync.dma_start(
                    out_d[eb:eb + R2 * 128, :].rearrange("(g p) m -> p g m", p=128),
                    acc[:, :R2])

            pool_q.append((0.0, k_now + 10, out_dma))

        # ---------------- the scheduled iterations
        for k in range(ntiles + 4):
            tb = k - 1           # backward tile (stages a-d)
            tf = k               # forward tile
            te = k - 2           # deferred stage_e / copies tile
            td = k - 1           # deferred dt tile
            tp = k - 3           # deferred postproc tile (prods/reduces)
            fw = tf < ntiles
            bw = 0 <= tb < ntiles
            ee = 0 <= te < ntiles
            dd = 0 <= td < ntiles
            pp = 0 <= tp < ntiles

            if fw: emit_fwd(tf, 0)                      # PE: a1 (+dma)
            if bw: bw_a(tb, 0); bw_a(tb, 1)             # PE: A0 A1
            if fw: emit_fwd(tf, 1)                      # Act: e1 lg1
            if bw: bw_a(tb, 2)                          # PE: A2
            if ee:
                bw_e(te, 0); bw_e(te, 1)                # PE (deferred)
                bw_e(te, 2); bw_e(te, 3)
            if fw: emit_fwd(tf, 2)                      # Act: r1 ; DVE: sg1
            if ee: emit_copies(te)                      # Pool
            if pp: emit_prods(tp)                       # DVE: gram / Pool: vprod
            if bw:
                bw_b(tb, 0); bw_b(tb, 1)                # DVE+Pool
                bw_b(tb, 2)
            if fw: emit_fwd(tf, 3)                      # PE: a2
            if bw: bw_c(tb, 0); bw_c(tb, 1)             # PE
            if dd: emit_fwd(td, 10)                     # PE: dt (deferred)
            if bw: bw_d(tb, 0)                          # DVE+Pool
            if fw: emit_fwd(tf, 4)                      # Act: e2 lg2 r2
            if fw: emit_fwd(tf, 71)                     # PE: a3 lg1-half
            if fw: emit_fwd(tf, 6)                      # DVE: sg2
            if fw: emit_fwd(tf, 5)                      # DVE: hm2
            if bw: bw_d(tb, 1)                          # DVE+Pool
            if fw: emit_fwd(tf, 72)                     # PE: a3 lg2-half
            if bw: bw_a(tb, 3)                          # PE: A3
            if bw: bw_b(tb, 3)                          # DVE+Pool
            if bw: bw_c(tb, 2); bw_c(tb, 3)             # PE
            if bw: bw_d(tb, 2); bw_d(tb, 3)             # DVE+Pool
            if pp: emit_reds(tp)                        # DVE: gred / Pool: vred
            if fw: emit_fwd(tf, 8)                      # Act: e3 lg3 r3
            if fw: emit_fwd(tf, 9)                      # DVE: hm3
            cur_k[0] = k
            drain_pool_q(2400)
            if te - 1 >= 0:
                st.pop(te - 1, None)

        while pool_q:
            cost, mink, fn = pool_q.popleft()
            fn()

    _split_multi_waits(nc)
    return nc


# ---------------------------------------------------------------- entry point
def kernel(zstates, W1, W2, W3, W4):
    from concourse.bass_utils import run_bass_kernel_spmd

    key = "full"
    if key not in _cache:
        _cache[key] = _build()
    nc = _cache[key]

    wm = _prep_weights(W1, W2, W3, W4)
    z = np.asarray(zstates, np.float32).reshape(NCORES, BP, NS)
    in_maps = [
        {**wm, "zT": np.ascontiguousarray(z[c].T)} for c in range(NCORES)
    ]
    res = run_bass_kernel_spmd(nc, in_maps, core_ids=list(range(NCORES)))
    return np.concatenate([res.results[c]["out"] for c in range(NCORES)], axis=0)


# revision 17
# speedup vs baseline: 1.1919x; 1.0087x over previous
"""nn_CoMet Trainium2 kernel (v2).

Math (per batch element s in R^16):
  MLP: h1 = logsig(s@W1.T); h2 = h1 + logsig(h1@W2.T); h3 = h2 + logsig(h2@W3.T)
       nnout = h3@W4.T ; d = nnout[:16]; com-rows C = W4[16:20]
  J = d com/d s  (reverse mode through the MLP, 4 rows)
  out = d - J^T (J J^T)^{-1} J d      (== the QR-based projection in the reference)

Kernel design (per core, batch 32768, hidden-major layout [hidden on partitions,
elements on free axis], tiles of 512 elements):
  - forward: activations via the natural_log_exp table only: e=Exp(-a),
    lg=Ln(e+1) (= -logsig(a) = hm increment), r=Exp(-lg) (= sigmoid(a));
    sg = 1-r on DVE tensor_scalar (4x mode).  a3 takes lg1 and lg2 as two
    separate moving passes so the spine never waits on the DVE hm2 add.
  - backward per com-row i: stage_a computes v = -C_i diag(r3) W3 into PSUM
    via fp8 DoubleRow matmuls (stationary -ASC*diag(C_i)W3, moving r3).  The
    residual bias B_i = ASC*(C_i + C_i@W3) is NOT materialized in PSUM;
    instead both PSUM reads apply it inline with scalar_tensor_tensor:
      v2s2 = (v + B_i) * sg2   (j-half 0 on DVE, half 1 on Pool, in parallel)
      v1s1 = (v + B_i) * sg1   (same split)
    bw_c accumulates W2^T v2s2 on top of the same PSUM.  The 1/ASC scale is
    folded into the bw_e stationary W1/ASC.
  - J and d produced transposed (element-major); Gram products on DVE (2x),
    Jd products on Pool; 4x4 normal-equation solve batched over 8 supertiles
    on Pool; combine on DVE.
"""

import numpy as np
import ml_dtypes

import concourse.bass as bass
import concourse.mybir as mybir
import concourse.tile as tile

NCORES = 8
B = 262144
BP = B // NCORES          # 32768 per core
NS = 16
NH = 256
NCOM = 4
TN = 512                  # elements per pipeline tile
NTILES = BP // TN         # 64
SUP = 2                   # tiles per supertile (postproc unit) -> 1024 elems
GRPS = SUP * TN // 128    # 8 groups of 128 elems per supertile
SOLVE_SUPS = 8            # supertiles per batched solve -> 8192 elems
SROWS = SOLVE_SUPS * GRPS # 64
ASC = 128.0               # backward fp8 scale (M3) ; removed via W1b

F32 = mybir.dt.float32
F32R = mybir.dt.float32r
BF16 = mybir.dt.bfloat16
F8E4 = mybir.dt.float8e4
AF = mybir.ActivationFunctionType
OP = mybir.AluOpType
AX = mybir.AxisListType

_cache = {}


# ---------------------------------------------------------------- drain patch
def _patch_tile_drain():
    """walrus in this container rejects >1 sem wait on the TileContext final
    drain ("Too many sync wait commands"); split the waits across several
    drain instructions (1 wait each)."""
    if getattr(tile.TileContext, "_comet_patched", False):
        return
    from concourse.vector_clock import ScopedClock

    def _drain_and_barrier(self, tick_clock, wait_clock):
        nc = self.nc
        drain_inst = nc.sync.drain()
        wait_clock.add_sem_waits(
            drain_inst.ins, ScopedClock({None: tick_clock.global_clock})
        )
        si = drain_inst.ins.sync_info
        waits = list(si.on_wait) if si is not None and si.on_wait else []
        if len(waits) > 1:
            si.on_wait = waits[:1]
            for w in waits[1:]:
                extra = nc.sync.drain()
                esi = extra.ins.sync_info
                if esi is None:
                    import bass_rust
                    extra.ins.sync_info = bass_rust.SyncInfo(
                        on_wait=[w], on_update=[]
                    )
                else:
                    esi.on_wait = list(esi.on_wait or []) + [w]
        nc.all_engine_barrier()
        assert self.sems is not None
        popped = nc._tile_sem_poison_stack.pop()
        assert popped is self._sem_poison
        nc.clear_and_free_semaphores(list(self.sems.allocated().values()))
        nc.all_engine_barrier()

    tile.TileContext._drain_and_barrier = _drain_and_barrier
    tile.TileContext._comet_patched = True


def _split_multi_waits(nc):
    """This container's walrus rejects instructions with more than one sync
    wait command.  Hoist extra waits onto injected same-engine NOPs placed
    immediately before the offending instruction."""
    import bass_rust

    for f in nc.m.functions:
        for b in f.blocks:
            insts = list(b.instructions)
            out, dirty = [], False
            for inst in insts:
                si = inst.sync_info
                waits = list(si.on_wait) if si is not None and si.on_wait else []
                if len(waits) > 1:
                    dirty = True
                    for k, wx in enumerate(waits[:-1]):
                        nop = mybir.InstNoOp(name=f"{inst.name}-ws{k}")
                        nop.engine = inst.engine
                        nop.sync_info = bass_rust.SyncInfo(
                            on_wait=[wx], on_update=[])
                        out.append(nop)
                    si.on_wait = waits[-1:]
                out.append(inst)
            if dirty:
                b.instructions = out


# ---------------------------------------------------------------- host prep
def _prep_weights(W1, W2, W3, W4):
    W1 = np.asarray(W1, np.float32)
    W2 = np.asarray(W2, np.float32)
    W3 = np.asarray(W3, np.float32)
    W4 = np.asarray(W4, np.float32)
    W4d, C = W4[:NS], W4[NS:]
    bf = ml_dtypes.bfloat16

    def khalf(a, m):   # [256, m] -> [128, 2, m]
        return np.ascontiguousarray(a.reshape(2, 128, m).transpose(1, 0, 2))

    f8 = ml_dtypes.float8_e4m3
    L1 = np.ascontiguousarray(W1.T)                       # [16,256] lhsT for a1
    L2 = khalf(-W2.T, NH).astype(bf)                      # [128,2,256]
    L3 = khalf(-W3.T, NH).astype(bf)
    L4 = khalf(-W4d.T, NS).astype(bf)                     # [128,2,16]
    # stage_a stationary: -diag(C_i)W3 (moving operand is r3 = sigmoid(a3)),
    # scaled by ASC and quantized to fp8 e4m3 for the DoubleRow matmul; the
    # scale is removed via the 1/ASC factor folded into W1b.
    M3 = np.stack([-C[i][:, None] * W3 for i in range(4)])  # [4,256,256]
    M3 = np.ascontiguousarray(
        (ASC * M3).reshape(4, 2, 128, NH).transpose(2, 1, 0, 3)  # [128,2,4,256]
    ).astype(f8)
    W2b = khalf(W2, NH).astype(bf)                        # [128,2,256] lhsT for U
    W1b = khalf(W1 / ASC, NS).astype(bf)                  # [128,2,16]  rhs for Jt
    # bias B_i = ASC*(C_i + C_i@W3): folds the residual C_i and the (1-r3)
    # complement of the layer-3 backward, injected into PSUM once per i by a
    # K=1 ones-matmul during stage_a; it then serves both backward stages.
    Bm = ASC * (C + C @ W3)                               # [4, 256]
    Cb = np.ascontiguousarray(Bm[:, None, :]).transpose(1, 0, 2)  # [1,4,256]
    return {
        "L1": L1, "L2": L2, "L3": L3, "L4": L4,
        "M3": M3, "W2b": W2b, "W1b": W1b, "Cb": Cb.astype(bf),
    }


# ---------------------------------------------------------------- build
def _build(ntiles=NTILES, a_bufs=1, v_bufs=2):
    _patch_tile_drain()
    nc = bass.Bass()

    zT = nc.dram_tensor("zT", [NS, BP], F32R, kind="ExternalInput")
    dL1 = nc.dram_tensor("L1", [NS, NH], F32R, kind="ExternalInput")
    dL2 = nc.dram_tensor("L2", [128, 2, NH], BF16, kind="ExternalInput")
    dL3 = nc.dram_tensor("L3", [128, 2, NH], BF16, kind="ExternalInput")
    dL4 = nc.dram_tensor("L4", [128, 2, NS], BF16, kind="ExternalInput")
    dM3 = nc.dram_tensor("M3", [128, 2, 4, NH], F8E4, kind="ExternalInput")
    dW2b = nc.dram_tensor("W2b", [128, 2, NH], BF16, kind="ExternalInput")
    dW1b = nc.dram_tensor("W1b", [128, 2, NS], BF16, kind="ExternalInput")
    dCb = nc.dram_tensor("Cb", [1, 4, NH], BF16, kind="ExternalInput")
    out_d = nc.dram_tensor("out", [BP, NS], F32, kind="ExternalOutput")

    from contextlib import ExitStack
    with tile.TileContext(nc) as tc, ExitStack() as ctx:
        wpool = ctx.enter_context(tc.tile_pool(name="w", bufs=1))
        sb = ctx.enter_context(tc.tile_pool(name="sb", bufs=3))
        sbe = ctx.enter_context(tc.tile_pool(name="sbe", bufs=6))
        sbv = ctx.enter_context(tc.tile_pool(name="sbv", bufs=6))
        sbz = ctx.enter_context(tc.tile_pool(name="sbz", bufs=6))
        jpool = ctx.enter_context(tc.tile_pool(name="jp", bufs=2))
        mpool = ctx.enter_context(tc.tile_pool(name="mp", bufs=2))
        aps_pool = ctx.enter_context(tc.tile_pool(name="aps", bufs=a_bufs, space="PSUM"))
        vps_pool = ctx.enter_context(tc.tile_pool(name="vps", bufs=v_bufs, space="PSUM"))
        tps_pool = ctx.enter_context(tc.tile_pool(name="tps", bufs=1, space="PSUM"))

        # ---- load constants
        L1s = wpool.tile([NS, NH], F32R)
        nc.sync.dma_start(L1s[:], dL1[:])
        L2s = wpool.tile([128, 2, NH], BF16)
        nc.sync.dma_start(L2s[:], dL2[:])
        L3s = wpool.tile([128, 2, NH], BF16)
        nc.sync.dma_start(L3s[:], dL3[:])
        L4s = wpool.tile([128, 2, NS], BF16)
        nc.sync.dma_start(L4s[:], dL4[:])
        M3s = wpool.tile([128, 2, 4, NH], F8E4)
        nc.sync.dma_start(M3s[:], dM3[:])
        W2s = wpool.tile([128, 2, NH], BF16)
        nc.sync.dma_start(W2s[:], dW2b[:])
        W1s = wpool.tile([128, 2, NS], BF16)
        nc.sync.dma_start(W1s[:], dW1b[:])
        Cbs = wpool.tile([1, 4, NH], BF16)
        nc.sync.dma_start(Cbs[:], dCb[:])
        ones = wpool.tile([1, TN], BF16)
        nc.vector.memset(ones[:], 1.0)

        # ---- hand-scheduled software pipeline.
        from collections import deque

        st = {}        # per-tile forward state
        sup = {}       # per-supertile PSUM accumulators
        grp = {"nsup": 0}
        pool_q = deque()   # (cost_ns, min_iter, closure) deferred solve/combine
        cur_k = [0]

        def drain_pool_q(budget):
            while pool_q and budget > 0:
                cost, mink, fn = pool_q[0]
                if mink > cur_k[0]:
                    break
                pool_q.popleft()
                fn()
                budget -= cost

        def emit_fwd(t, part):
            S = st.setdefault(t, {})
            w, s = t % SUP, t // SUP
            if part == 0:      # dma + a1
                if w == 0:
                    sup[s] = {
                        "jt": tps_pool.tile([128, GRPS, 4 * NS], F32, tag="jtps", name="jt_ps"),
                        "dt": tps_pool.tile([128, GRPS, NS], F32, tag="dtps", name="dt_ps"),
                    }
                zt = sbz.tile([NS, TN], F32R, tag="zt")
                nc.sync.dma_start(zt[:], zT[:, t * TN:(t + 1) * TN])
                a_ps = aps_pool.tile([128, 2, TN], F32, tag="aps")
                for j in range(2):
                    nc.tensor.matmul(a_ps[:, j], L1s[:, j * 128:(j + 1) * 128],
                                     zt[:], start=True, stop=True)
                S["a"] = a_ps
            elif part == 1:    # e1, lg1  (Act)
                e = sbe.tile([128, 2, TN], BF16, tag="e")
                nc.scalar.activation(e[:], S["a"][:], AF.Exp, scale=-1.0)
                lg1 = sb.tile([128, 2, TN], BF16, tag="lg1")
                nc.scalar.activation(lg1[:], e[:], AF.Ln, bias=1.0)
                S["lg1"] = lg1
            elif part == 2:    # r1 (Act) + sg1 (Pool)
                r1 = sb.tile([128, 2, TN], BF16, tag="r1")
                nc.scalar.activation(r1[:], S["lg1"][:], AF.Exp, scale=-1.0)
                s1 = sb.tile([128, 2, TN], BF16, tag="s1")
                nc.gpsimd.tensor_scalar(s1[:], r1[:], -1.0, 1.0,
                                        OP.mult, OP.add)
                S["s1"] = s1
            elif part == 3:    # a2
                a_ps = aps_pool.tile([128, 2, TN], F32, tag="aps")
                for j in range(2):
                    for k in range(2):
                        nc.tensor.matmul(a_ps[:, j],
                                         L2s[:, k, j * 128:(j + 1) * 128],
                                         S["lg1"][:, k], start=(k == 0), stop=(k == 1))
                S["a"] = a_ps
            elif part == 4:    # e2, lg2, r2 (Act)
                e = sbe.tile([128, 2, TN], BF16, tag="e")
                nc.scalar.activation(e[:], S["a"][:], AF.Exp, scale=-1.0)
                lg2 = sb.tile([128, 2, TN], BF16, tag="lg2")
                nc.scalar.activation(lg2[:], e[:], AF.Ln, bias=1.0)
                S["lg2"] = lg2
                r2 = sb.tile([128, 2, TN], BF16, tag="r2")
                nc.scalar.activation(r2[:], lg2[:], AF.Exp, scale=-1.0)
                S["r2"] = r2
            elif part == 6:    # sg2 (Pool)
                s2 = sb.tile([128, 2, TN], BF16, tag="s2")
                nc.gpsimd.tensor_scalar(s2[:], S["r2"][:], -1.0, 1.0,
                                        OP.mult, OP.add)
                S["s2"] = s2
            elif part == 71:   # a3 first half: W3 @ lg1 (spine-free)
                a_ps = aps_pool.tile([128, 2, TN], F32, tag="aps")
                for j in range(2):
                    for k in range(2):
                        nc.tensor.matmul(a_ps[:, j],
                                         L3s[:, k, j * 128:(j + 1) * 128],
                                         S["lg1"][:, k], start=(k == 0), stop=False)
                S["a"] = a_ps
            elif part == 72:   # a3 second half: W3 @ lg2 (waits only on lg2)
                a_ps = S["a"]
                for j in range(2):
                    for k in range(2):
                        nc.tensor.matmul(a_ps[:, j],
                                         L3s[:, k, j * 128:(j + 1) * 128],
                                         S["lg2"][:, k], start=False, stop=(k == 1))
            elif part == 8:    # e3, lg3, r3 (Act)
                e = sbe.tile([128, 2, TN], BF16, tag="e")
                nc.scalar.activation(e[:], S["a"][:], AF.Exp, scale=-1.0)
                lg3 = sb.tile([128, 2, TN], BF16, tag="lg3")
                nc.scalar.activation(lg3[:], e[:], AF.Ln, bias=1.0)
                S["lg3"] = lg3
                r3 = sb.tile([128, 2, TN], F8E4, tag="r3")
                nc.scalar.activation(r3[:], lg3[:], AF.Exp, scale=-1.0)
                S["r3"] = r3
            elif part == 10:   # dt matmuls: d = W4d^T (lg1 + lg2 + lg3), 3 passes
                dt_ps = sup[s]["dt"]
                lgs = (S["lg1"], S["lg2"], S["lg3"])
                for g in range(4):
                    gi = w * 4 + g
                    for li, lg in enumerate(lgs):
                        for k in range(2):
                            nc.tensor.matmul(dt_ps[:, gi],
                                             lg[:, k, g * 128:(g + 1) * 128],
                                             L4s[:, k],
                                             start=(li == 0 and k == 0),
                                             stop=(li == 2 and k == 1))

        def bw_a(t, i):
            S = st[t]
            v_ps = vps_pool.tile([128, 2, TN], F32, tag="vps", name=f"vps{i}")
            S.setdefault("v", {})[i] = v_ps
            for j in range(2):
                nc.tensor.matmul(v_ps[:, j],
                                 M3s[:, :, i, j * 128:(j + 1) * 128],
                                 S["r3"][:], start=True, stop=False,
                                 perf_mode=mybir.MatmulPerfMode.DoubleRow)
                nc.tensor.matmul(v_ps[:, j], Cbs[:, i, j * 128:(j + 1) * 128],
                                 ones[:], start=False, stop=False)

        def bw_b(t, i):
            S = st[t]
            v2s2 = sbv.tile([128, 2, TN], BF16, tag="v2s2", name=f"v2s2_{i}")
            S.setdefault("v2", {})[i] = v2s2
            nc.vector.tensor_tensor(v2s2[:], S["v"][i][:], S["s2"][:], OP.mult)

        def bw_c(t, i):
            S = st[t]
            for j in range(2):
                for k in range(2):
                    nc.tensor.matmul(S["v"][i][:, j],
                                     W2s[:, k, j * 128:(j + 1) * 128],
                                     S["v2"][i][:, k], start=False, stop=(k == 1))

        def bw_d(t, i):
            S = st[t]
            v1s1 = sbv.tile([128, 2, TN], BF16, tag="v1s1", name=f"v1s1_{i}")
            S.setdefault("v1", {})[i] = v1s1
            nc.vector.tensor_tensor(v1s1[:], S["v"][i][:], S["s1"][:], OP.mult)

        def bw_e(t, i):
            S = st[t]
            w = t % SUP
            jt_ps = sup[t // SUP]["jt"]
            for g in range(4):
                gi = w * 4 + g
                for k in range(2):
                    nc.tensor.matmul(jt_ps[:, gi, i * NS:(i + 1) * NS],
                                     S["v1"][i][:, k, g * 128:(g + 1) * 128],
                                     W1s[:, k], start=(k == 0), stop=(k == 1))

        def emit_copies(t):
            # per-supertile: copy jt/dt accumulators out of PSUM (Pool)
            w, s = t % SUP, t // SUP
            if w != SUP - 1:
                return
            sq = s % SOLVE_SUPS
            if sq == 0:
                grp["msb"] = mpool.tile([128, 10, SROWS], F32, tag="msb", name="msb")
                grp["vsb"] = mpool.tile([128, 4, SROWS], F32, tag="vsb", name="vsb")
                grp["csb"] = mpool.tile([128, SROWS, 4], F32, tag="csb", name="csb")
                grp["jtg"] = jpool.tile([128, SROWS, 4 * NS], BF16, tag="jtg", name="jtg")
                grp["dtg"] = jpool.tile([128, SROWS, NS], F32, tag="dtg", name="dtg")
                grp["nsup"] = 0
            rows = slice(sq * GRPS, (sq + 1) * GRPS)
            nc.scalar.copy(grp["jtg"][:, rows], sup[s]["jt"][:])
            nc.scalar.copy(grp["dtg"][:, rows], sup[s]["dt"][:])
            grp["nsup"] += 1
            sup[s]["jtg"] = grp["jtg"]
            sup[s]["dtg"] = grp["dtg"]
            sup[s]["msb"] = grp["msb"]
            sup[s]["vsb"] = grp["vsb"]
            sup[s]["csb"] = grp["csb"]
            sup[s]["sq"] = sq
            sup[s]["nsup"] = grp["nsup"]

        def emit_prods(t):
            # deferred products for the supertile ending at tile t
            w, s = t % SUP, t // SUP
            if w != SUP - 1:
                return
            sq = sup[s]["sq"]
            rows = slice(sq * GRPS, (sq + 1) * GRPS)
            jts = sup[s]["jtg"][:, rows]
            dts = sup[s]["dtg"][:, rows]
            gprod = sb.tile([128, 10, GRPS, NS], BF16, tag="gprod")
            pbase = 0
            for a in range(4):
                nb = 4 - a
                in0 = jts[:, :, a * NS:(a + 1) * NS].unsqueeze(1) \
                    .to_broadcast((128, nb, GRPS, NS))
                in1 = jts[:, :, a * NS:].rearrange("p g (b n) -> p b g n", b=nb)
                nc.gpsimd.tensor_tensor(gprod[:, pbase:pbase + nb],
                                        in0, in1, OP.mult)
                pbase += nb
            vprod = sb.tile([128, 4, GRPS, NS], F32, tag="vprod")
            nc.gpsimd.tensor_tensor(
                vprod[:],
                jts[:].rearrange("p g (b n) -> p b g n", b=4),
                dts[:].unsqueeze(1).to_broadcast((128, 4, GRPS, NS)),
                OP.mult)
            sup[s]["gprod"] = gprod
            sup[s]["vprod"] = vprod

        def emit_reds(t):
            # deferred reductions for the supertile ending at tile t
            w, s = t % SUP, t // SUP
            if w != SUP - 1:
                return
            sq = sup[s]["sq"]
            rows = slice(sq * GRPS, (sq + 1) * GRPS)
            nc.vector.tensor_reduce(
                sup[s]["msb"][:, :, rows].unsqueeze(3), sup[s]["gprod"][:],
                AX.X, OP.add)
            nc.vector.tensor_reduce(
                sup[s]["vsb"][:, :, rows].unsqueeze(3), sup[s]["vprod"][:],
                AX.X, OP.add)
            if sup[s]["sq"] == SOLVE_SUPS - 1 or t == ntiles - 1:
                enqueue_solve(s)

        def enqueue_solve(s):
            msb, vsb, csb = sup[s]["msb"], sup[s]["vsb"], sup[s]["csb"]
            jtg, dtg = sup[s]["jtg"], sup[s]["dtg"]
            nsup = sup[s]["nsup"]
            R = nsup * GRPS

            def m(i_):
                return msb[:, i_, :R].unsqueeze(2)

            def vv(i_):
                return vsb[:, i_, :R].unsqueeze(2)

            tt = {}

            def tmp(name):
                if name not in tt:
                    tt[name] = mpool.tile([128, SROWS, 1], F32, tag=f"t_{name}", name=f"t_{name}")
                return tt[name][:, :R]

            V = nc.gpsimd
            step_cost = 64 * 1.39 + 95
            k_now = cur_k[0]

            def q(*ops, lag=1):
                for op in ops:
                    pool_q.append((step_cost, k_now + lag, op))

            def mul(o, x, y):
                return lambda: V.tensor_tensor(o, x, y, OP.mult)

            def sub(o, x, y):
                return lambda: V.tensor_tensor(o, x, y, OP.subtract)

            def add(o, x, y):
                return lambda: V.tensor_tensor(o, x, y, OP.add)

            # lazily build AP closures (tiles exist already)
            m00, m01, m02, m03, m11, m12, m13, m22, m23, m33 = (m(i_) for i_ in range(10))
            x1, x2 = tmp("x1"), tmp("x2")
            detA, u10, u11 = tmp("detA"), tmp("u10"), tmp("u11")
            P00, P01, P10, P11 = tmp("P00"), tmp("P01"), tmp("P10"), tmp("P11")
            S00, S01, S11 = tmp("S00"), tmp("S01"), tmp("S11")
            w0, w1, detS = tmp("w0"), tmp("w1"), tmp("detS")
            cw2, cw3, q0, q1 = tmp("cw2"), tmp("cw3"), tmp("q0"), tmp("q1")
            dAS, rAS, rS = tmp("dAS"), tmp("rAS"), tmp("rS")
            x3, x4 = tmp("x3"), tmp("x4")

            q(mul(x1, m00, m11), mul(x2, m01, m01), sub(detA, x1, x2))
            q(mul(x1, m11, vv(0)), mul(x2, m01, vv(1)), sub(u10, x1, x2))
            q(mul(x1, m00, vv(1)), mul(x2, m01, vv(0)), sub(u11, x1, x2))
            q(mul(x1, m11, m02), mul(x2, m01, m12), sub(P00, x1, x2))
            q(mul(x1, m11, m03), mul(x2, m01, m13), sub(P01, x1, x2))
            q(mul(x1, m00, m12), mul(x2, m01, m02), sub(P10, x1, x2))
            q(mul(x1, m00, m13), mul(x2, m01, m03), sub(P11, x1, x2))
            q(mul(x1, m02, P00), mul(x2, m12, P10), add(x1, x1, x2))
            q(mul(S00, detA, m22), sub(S00, S00, x1))
            q(mul(x3, m02, P01), mul(x4, m12, P11), add(x3, x3, x4))
            q(mul(S01, detA, m23), sub(S01, S01, x3))
            q(mul(x1, m03, P01), mul(x2, m13, P11), add(x1, x1, x2))
            q(mul(S11, detA, m33), sub(S11, S11, x1))
            q(mul(x3, m02, u10), mul(x4, m12, u11), add(x3, x3, x4))
            q(mul(w0, detA, vv(2)), sub(w0, w0, x3))
            q(mul(x1, m03, u10), mul(x2, m13, u11), add(x1, x1, x2))
            q(mul(w1, detA, vv(3)), sub(w1, w1, x1))
            q(mul(x3, S00, S11), mul(x4, S01, S01), sub(detS, x3, x4))
            q(mul(x1, S11, w0), mul(x2, S01, w1), sub(cw2, x1, x2))
            q(mul(x3, S00, w1), mul(x4, S01, w0), sub(cw3, x3, x4))
            q(mul(x1, P00, cw2), mul(x2, P01, cw3), add(x1, x1, x2))
            q(mul(q0, u10, detS), sub(q0, q0, x1))
            q(mul(x3, P10, cw2), mul(x4, P11, cw3), add(x3, x3, x4))
            q(mul(q1, u11, detS), sub(q1, q1, x3))
            q(mul(dAS, detA, detS))
            q(lambda: nc.vector.reciprocal(rAS, dAS), lag=5)
            q(lambda: nc.vector.reciprocal(rS, detS), lag=5)
            q(mul(csb[:, :R, 0:1], q0, rAS), mul(csb[:, :R, 1:2], q1, rAS), lag=5)
            q(mul(csb[:, :R, 2:3], cw2, rS), mul(csb[:, :R, 3:4], cw3, rS), lag=5)

            # combine + output DMA on DVE, sliced into quarter-row chunks
            s_base = s - (nsup - 1)
            R2 = nsup * GRPS
            acc = sb.tile([128, SROWS, NS], F32, tag="acc")
            ctmp = sb.tile([128, SROWS, NS], F32, tag="ctmp")
            RQ = R2 // 4
            slice_cost = RQ * NS * 1.99 + 95

            def comb(a, r0, r1):
                def fn():
                    cb = csb[:, r0:r1, a:a + 1].to_broadcast((128, r1 - r0, NS))
                    V.tensor_tensor(ctmp[:, r0:r1], cb,
                                    jtg[:, r0:r1, a * NS:(a + 1) * NS], OP.mult)
                    if a == 0:
                        V.tensor_tensor(acc[:, r0:r1], dtg[:, r0:r1],
                                        ctmp[:, r0:r1], OP.subtract)
                    else:
                        V.tensor_tensor(acc[:, r0:r1], acc[:, r0:r1],
                                        ctmp[:, r0:r1], OP.subtract)
                return fn

            for a in range(4):
                for rq in range(4):
                    pool_q.append((2 * slice_cost, k_now + 5 + a,
                                   comb(a, rq * RQ, (rq + 1) * RQ)))

            def out_dma():
                eb = s_base * SUP * TN
                nc.sync.dma_start(
                    out_d[eb:eb + R2 * 128, :].rearrange("(g p) m -> p g m", p=128),
                    acc[:, :R2])

            pool_q.append((0.0, k_now + 10, out_dma))

        # ---------------- the scheduled iterations
        for k in range(ntiles + 4):
            tb = k - 1           # backward tile (stages a-d)
            tf = k               # forward tile
            te = k - 2           # deferred stage_e / copies tile
            td = k - 1           # deferred dt tile
            tp = k - 3           # deferred postproc tile (prods/reduces)
            fw = tf < ntiles
            bw = 0 <= tb < ntiles
            ee = 0 <= te < ntiles
            dd = 0 <= td < ntiles
            pp = 0 <= tp < ntiles

            if fw: emit_fwd(tf, 0)                      # PE: a1 (+dma)
            if bw: bw_a(tb, 0); bw_a(tb, 1)             # PE: A0 A1
            if fw: emit_fwd(tf, 1)                      # Act: e1 lg1
            if ee:
                bw_e(te, 0); bw_e(te, 1)                # PE (deferred)
                bw_e(te, 2); bw_e(te, 3)
            if bw: bw_b(tb, 0); bw_b(tb, 1)             # DVE: B0 B1
            if fw: emit_fwd(tf, 2)                      # Act: r1 ; Pool: sg1
            if ee: emit_copies(te)                      # Act (fills a2-wait gap)
            if bw: bw_c(tb, 0)                          # PE: C0
            if fw: emit_fwd(tf, 3)                      # PE: a2
            if bw: bw_d(tb, 0)                          # DVE: D0
            if bw: bw_c(tb, 1)                          # PE: C1
            if dd: emit_fwd(td, 10)                     # PE: dt (deferred)
            if bw: bw_d(tb, 1)                          # DVE: D1
            if bw: bw_a(tb, 2)                          # PE: A2 (after D0)
            if fw: emit_fwd(tf, 4)                      # Act: e2 lg2 r2
            if fw: emit_fwd(tf, 71)                     # PE: a3 lg1-half
            if fw: emit_fwd(tf, 6)                      # Pool: sg2
            if bw: bw_b(tb, 2)                          # DVE: B2
            if pp: emit_prods(tp)                       # Pool: gram/vprod
            if fw: emit_fwd(tf, 72)                     # PE: a3 lg2-half
            if bw: bw_a(tb, 3)                          # PE: A3 (after D1)
            if bw: bw_b(tb, 3)                          # DVE: B3
            if bw: bw_c(tb, 2)                          # PE: C2
            if bw: bw_d(tb, 2)                          # DVE: D2
            if bw: bw_c(tb, 3)                          # PE: C3
            if bw: bw_d(tb, 3)                          # DVE: D3
            if pp: emit_reds(tp)                        # DVE (+ enqueue solve)
            if fw: emit_fwd(tf, 8)                      # Act: e3 lg3 r3
            cur_k[0] = k
            drain_pool_q(2400)
            if te - 1 >= 0:
                st.pop(te - 1, None)

        while pool_q:
            cost, mink, fn = pool_q.popleft()
            fn()

    _split_multi_waits(nc)
    return nc


# ---------------------------------------------------------------- entry point
def kernel(zstates, W1, W2, W3, W4):
    from concourse.bass_utils import run_bass_kernel_spmd

    key = "full"
    if key not in _cache:
        _cache[key] = _build()
    nc = _cache[key]

    wm = _prep_weights(W1, W2, W3, W4)
    z = np.asarray(zstates, np.float32).reshape(NCORES, BP, NS)
    in_maps = [
        {**wm, "zT": np.ascontiguousarray(z[c].T)} for c in range(NCORES)
    ]
    res = run_bass_kernel_spmd(nc, in_maps, core_ids=list(range(NCORES)))
    return np.concatenate([res.results[c]["out"] for c in range(NCORES)], axis=0)


# revision 24
# speedup vs baseline: 1.1947x; 1.0024x over previous
"""nn_CoMet Trainium2 kernel (v2).

Math (per batch element s in R^16):
  MLP: h1 = logsig(s@W1.T); h2 = h1 + logsig(h1@W2.T); h3 = h2 + logsig(h2@W3.T)
       nnout = h3@W4.T ; d = nnout[:16]; com-rows C = W4[16:20]
  J = d com/d s  (reverse mode through the MLP, 4 rows)
  out = d - J^T (J J^T)^{-1} J d      (== the QR-based projection in the reference)

Kernel design (per core, batch 32768, hidden-major layout [hidden on partitions,
elements on free axis], tiles of 512 elements):
  - forward: activations via the natural_log_exp table only: e=Exp(-a),
    lg=Ln(e+1) (= -logsig(a) = hm increment), r=Exp(-lg) (= sigmoid(a));
    sg = 1-r on DVE tensor_scalar (4x mode).  a3 takes lg1 and lg2 as two
    separate moving passes so the spine never waits on the DVE hm2 add.
  - backward per com-row i: stage_a computes v = -C_i diag(r3) W3 into PSUM
    via fp8 DoubleRow matmuls (stationary -ASC*diag(C_i)W3, moving r3).  The
    residual bias B_i = ASC*(C_i + C_i@W3) is NOT materialized in PSUM;
    instead both PSUM reads apply it inline with scalar_tensor_tensor:
      v2s2 = (v + B_i) * sg2   (j-half 0 on DVE, half 1 on Pool, in parallel)
      v1s1 = (v + B_i) * sg1   (same split)
    bw_c accumulates W2^T v2s2 on top of the same PSUM.  The 1/ASC scale is
    folded into the bw_e stationary W1/ASC.
  - J and d produced transposed (element-major); Gram products on DVE (2x),
    Jd products on Pool; 4x4 normal-equation solve batched over 8 supertiles
    on Pool; combine on DVE.
"""

import numpy as np
import ml_dtypes

import concourse.bass as bass
import concourse.mybir as mybir
import concourse.tile as tile

NCORES = 8
B = 262144
BP = B // NCORES          # 32768 per core
NS = 16
NH = 256
NCOM = 4
TN = 512                  # elements per pipeline tile
NTILES = BP // TN         # 64
SUP = 2                   # tiles per supertile (postproc unit) -> 1024 elems
GRPS = SUP * TN // 128    # 8 groups of 128 elems per supertile
SOLVE_SUPS = 8            # supertiles per batched solve -> 8192 elems
SROWS = SOLVE_SUPS * GRPS # 64
ASC = 128.0               # backward fp8 scale (M3) ; removed via W1b

F32 = mybir.dt.float32
F32R = mybir.dt.float32r
BF16 = mybir.dt.bfloat16
F8E4 = mybir.dt.float8e4
AF = mybir.ActivationFunctionType
OP = mybir.AluOpType
AX = mybir.AxisListType

_cache = {}


# ---------------------------------------------------------------- drain patch
def _patch_tile_drain():
    """walrus in this container rejects >1 sem wait on the TileContext final
    drain ("Too many sync wait commands"); split the waits across several
    drain instructions (1 wait each)."""
    if getattr(tile.TileContext, "_comet_patched", False):
        return
    from concourse.vector_clock import ScopedClock

    def _drain_and_barrier(self, tick_clock, wait_clock):
        nc = self.nc
        drain_inst = nc.sync.drain()
        wait_clock.add_sem_waits(
            drain_inst.ins, ScopedClock({None: tick_clock.global_clock})
        )
        si = drain_inst.ins.sync_info
        waits = list(si.on_wait) if si is not None and si.on_wait else []
        if len(waits) > 1:
            si.on_wait = waits[:1]
            for w in waits[1:]:
                extra = nc.sync.drain()
                esi = extra.ins.sync_info
                if esi is None:
                    import bass_rust
                    extra.ins.sync_info = bass_rust.SyncInfo(
                        on_wait=[w], on_update=[]
                    )
                else:
                    esi.on_wait = list(esi.on_wait or []) + [w]
        nc.all_engine_barrier()
        assert self.sems is not None
        popped = nc._tile_sem_poison_stack.pop()
        assert popped is self._sem_poison
        nc.clear_and_free_semaphores(list(self.sems.allocated().values()))
        nc.all_engine_barrier()

    tile.TileContext._drain_and_barrier = _drain_and_barrier
    tile.TileContext._comet_patched = True


def _split_multi_waits(nc):
    """This container's walrus rejects instructions with more than one sync
    wait command.  Hoist extra waits onto injected same-engine NOPs placed
    immediately before the offending instruction."""
    import bass_rust

    for f in nc.m.functions:
        for b in f.blocks:
            insts = list(b.instructions)
            out, dirty = [], False
            for inst in insts:
                si = inst.sync_info
                waits = list(si.on_wait) if si is not None and si.on_wait else []
                if len(waits) > 1:
                    dirty = True
                    for k, wx in enumerate(waits[:-1]):
                        nop = mybir.InstNoOp(name=f"{inst.name}-ws{k}")
                        nop.engine = inst.engine
                        nop.sync_info = bass_rust.SyncInfo(
                            on_wait=[wx], on_update=[])
                        out.append(nop)
                    si.on_wait = waits[-1:]
                out.append(inst)
            if dirty:
                b.instructions = out


# ---------------------------------------------------------------- host prep
def _prep_weights(W1, W2, W3, W4):
    W1 = np.asarray(W1, np.float32)
    W2 = np.asarray(W2, np.float32)
    W3 = np.asarray(W3, np.float32)
    W4 = np.asarray(W4, np.float32)
    W4d, C = W4[:NS], W4[NS:]
    bf = ml_dtypes.bfloat16

    def khalf(a, m):   # [256, m] -> [128, 2, m]
        return np.ascontiguousarray(a.reshape(2, 128, m).transpose(1, 0, 2))

    f8 = ml_dtypes.float8_e4m3
    L1 = np.ascontiguousarray(W1.T)                       # [16,256] lhsT for a1
    L2 = khalf(-W2.T, NH).astype(bf)                      # [128,2,256]
    L3 = khalf(-W3.T, NH).astype(bf)
    L4 = khalf(-W4d.T, NS).astype(bf)                     # [128,2,16]
    # stage_a stationary: -diag(C_i)W3 (moving operand is r3 = sigmoid(a3)),
    # scaled by ASC and quantized to fp8 e4m3 for the DoubleRow matmul; the
    # scale is removed via the 1/ASC factor folded into W1b.
    M3 = np.stack([-C[i][:, None] * W3 for i in range(4)])  # [4,256,256]
    M3 = np.ascontiguousarray(
        (ASC * M3).reshape(4, 2, 128, NH).transpose(2, 1, 0, 3)  # [128,2,4,256]
    ).astype(f8)
    W2b = khalf(W2, NH).astype(bf)                        # [128,2,256] lhsT for U
    W1b = khalf(W1 / ASC, NS).astype(bf)                  # [128,2,16]  rhs for Jt
    # bias B_i = ASC*(C_i + C_i@W3): folds the residual C_i and the (1-r3)
    # complement of the layer-3 backward, injected into PSUM once per i by a
    # K=1 ones-matmul during stage_a; it then serves both backward stages.
    Bm = ASC * (C + C @ W3)                               # [4, 256]
    Cb = np.ascontiguousarray(Bm[:, None, :]).transpose(1, 0, 2)  # [1,4,256]
    return {
        "L1": L1, "L2": L2, "L3": L3, "L4": L4,
        "M3": M3, "W2b": W2b, "W1b": W1b, "Cb": Cb.astype(bf),
    }


# ---------------------------------------------------------------- build
def _build(ntiles=NTILES, a_bufs=1, v_bufs=2):
    _patch_tile_drain()
    nc = bass.Bass()

    zT = nc.dram_tensor("zT", [NS, BP], F32R, kind="ExternalInput")
    dL1 = nc.dram_tensor("L1", [NS, NH], F32R, kind="ExternalInput")
    dL2 = nc.dram_tensor("L2", [128, 2, NH], BF16, kind="ExternalInput")
    dL3 = nc.dram_tensor("L3", [128, 2, NH], BF16, kind="ExternalInput")
    dL4 = nc.dram_tensor("L4", [128, 2, NS], BF16, kind="ExternalInput")
    dM3 = nc.dram_tensor("M3", [128, 2, 4, NH], F8E4, kind="ExternalInput")
    dW2b = nc.dram_tensor("W2b", [128, 2, NH], BF16, kind="ExternalInput")
    dW1b = nc.dram_tensor("W1b", [128, 2, NS], BF16, kind="ExternalInput")
    dCb = nc.dram_tensor("Cb", [1, 4, NH], BF16, kind="ExternalInput")
    out_d = nc.dram_tensor("out", [BP, NS], F32, kind="ExternalOutput")

    from contextlib import ExitStack
    with tile.TileContext(nc) as tc, ExitStack() as ctx:
        wpool = ctx.enter_context(tc.tile_pool(name="w", bufs=1))
        sb = ctx.enter_context(tc.tile_pool(name="sb", bufs=3))
        sbe = ctx.enter_context(tc.tile_pool(name="sbe", bufs=6))
        sbv = ctx.enter_context(tc.tile_pool(name="sbv", bufs=6))
        sbz = ctx.enter_context(tc.tile_pool(name="sbz", bufs=6))
        jpool = ctx.enter_context(tc.tile_pool(name="jp", bufs=3))
        mpool = ctx.enter_context(tc.tile_pool(name="mp", bufs=3))
        aps_pool = ctx.enter_context(tc.tile_pool(name="aps", bufs=a_bufs, space="PSUM"))
        vps_pool = ctx.enter_context(tc.tile_pool(name="vps", bufs=v_bufs, space="PSUM"))
        tps_pool = ctx.enter_context(tc.tile_pool(name="tps", bufs=1, space="PSUM"))

        # ---- load constants
        L1s = wpool.tile([NS, NH], F32R)
        nc.sync.dma_start(L1s[:], dL1[:])
        L2s = wpool.tile([128, 2, NH], BF16)
        nc.sync.dma_start(L2s[:], dL2[:])
        L3s = wpool.tile([128, 2, NH], BF16)
        nc.sync.dma_start(L3s[:], dL3[:])
        L4s = wpool.tile([128, 2, NS], BF16)
        nc.sync.dma_start(L4s[:], dL4[:])
        M3s = wpool.tile([128, 2, 4, NH], F8E4)
        nc.sync.dma_start(M3s[:], dM3[:])
        W2s = wpool.tile([128, 2, NH], BF16)
        nc.sync.dma_start(W2s[:], dW2b[:])
        W1s = wpool.tile([128, 2, NS], BF16)
        nc.sync.dma_start(W1s[:], dW1b[:])
        Cbs = wpool.tile([1, 4, NH], BF16)
        nc.sync.dma_start(Cbs[:], dCb[:])
        ones = wpool.tile([1, TN], BF16)
        nc.vector.memset(ones[:], 1.0)

        # ---- hand-scheduled software pipeline.
        from collections import deque

        st = {}        # per-tile forward state
        sup = {}       # per-supertile PSUM accumulators
        grp = {"nsup": 0}
        pool_q = deque()   # (cost_ns, min_iter, closure) deferred solve/combine
        cur_k = [0]

        def drain_pool_q(budget):
            while pool_q and budget > 0:
                cost, mink, fn = pool_q[0]
                if mink > cur_k[0]:
                    break
                pool_q.popleft()
                fn()
                budget -= cost

        def emit_fwd(t, part):
            S = st.setdefault(t, {})
            w, s = t % SUP, t // SUP
            if part == 0:      # dma + a1
                if w == 0:
                    sup[s] = {
                        "jt": tps_pool.tile([128, GRPS, 4 * NS], F32, tag="jtps", name="jt_ps"),
                        "dt": tps_pool.tile([128, GRPS, NS], F32, tag="dtps", name="dt_ps"),
                    }
                zt = sbz.tile([NS, TN], F32R, tag="zt")
                nc.sync.dma_start(zt[:], zT[:, t * TN:(t + 1) * TN])
                a_ps = aps_pool.tile([128, 2, TN], F32, tag="aps")
                for j in range(2):
                    nc.tensor.matmul(a_ps[:, j], L1s[:, j * 128:(j + 1) * 128],
                                     zt[:], start=True, stop=True)
                S["a"] = a_ps
            elif part == 1:    # e1, lg1  (Act)
                e = sbe.tile([128, 2, TN], BF16, tag="e")
                nc.scalar.activation(e[:], S["a"][:], AF.Exp, scale=-1.0)
                lg1 = sb.tile([128, 2, TN], BF16, tag="lg1")
                nc.scalar.activation(lg1[:], e[:], AF.Ln, bias=1.0)
                S["lg1"] = lg1
            elif part == 2:    # r1 (Act) + sg1 (Pool)
                r1 = sb.tile([128, 2, TN], BF16, tag="r1")
                nc.scalar.activation(r1[:], S["lg1"][:], AF.Exp, scale=-1.0)
                s1 = sb.tile([128, 2, TN], BF16, tag="s1")
                nc.gpsimd.tensor_scalar(s1[:], r1[:], -1.0, 1.0,
                                        OP.mult, OP.add)
                S["s1"] = s1
            elif part == 3:    # a2
                a_ps = aps_pool.tile([128, 2, TN], F32, tag="aps")
                for j in range(2):
                    for k in range(2):
                        nc.tensor.matmul(a_ps[:, j],
                                         L2s[:, k, j * 128:(j + 1) * 128],
                                         S["lg1"][:, k], start=(k == 0), stop=(k == 1))
                S["a"] = a_ps
            elif part == 4:    # e2, lg2, r2 (Act)
                e = sbe.tile([128, 2, TN], BF16, tag="e")
                nc.scalar.activation(e[:], S["a"][:], AF.Exp, scale=-1.0)
                lg2 = sb.tile([128, 2, TN], BF16, tag="lg2")
                nc.scalar.activation(lg2[:], e[:], AF.Ln, bias=1.0)
                S["lg2"] = lg2
                r2 = sb.tile([128, 2, TN], BF16, tag="r2")
                nc.scalar.activation(r2[:], lg2[:], AF.Exp, scale=-1.0)
                S["r2"] = r2
            elif part == 6:    # sg2 (Pool)
                s2 = sb.tile([128, 2, TN], BF16, tag="s2")
                nc.gpsimd.tensor_scalar(s2[:], S["r2"][:], -1.0, 1.0,
                                        OP.mult, OP.add)
                S["s2"] = s2
            elif part == 71:   # a3 first half: W3 @ lg1 (spine-free)
                a_ps = aps_pool.tile([128, 2, TN], F32, tag="aps")
                for j in range(2):
                    for k in range(2):
                        nc.tensor.matmul(a_ps[:, j],
                                         L3s[:, k, j * 128:(j + 1) * 128],
                                         S["lg1"][:, k], start=(k == 0), stop=False)
                S["a"] = a_ps
            elif part == 72:   # a3 second half: W3 @ lg2 (waits only on lg2)
                a_ps = S["a"]
                for j in range(2):
                    for k in range(2):
                        nc.tensor.matmul(a_ps[:, j],
                                         L3s[:, k, j * 128:(j + 1) * 128],
                                         S["lg2"][:, k], start=False, stop=(k == 1))
            elif part == 8:    # e3, lg3, r3 (Act)
                e = sbe.tile([128, 2, TN], BF16, tag="e")
                nc.scalar.activation(e[:], S["a"][:], AF.Exp, scale=-1.0)
                lg3 = sb.tile([128, 2, TN], BF16, tag="lg3")
                nc.scalar.activation(lg3[:], e[:], AF.Ln, bias=1.0)
                S["lg3"] = lg3
                r3 = sb.tile([128, 2, TN], F8E4, tag="r3")
                nc.scalar.activation(r3[:], lg3[:], AF.Exp, scale=-1.0)
                S["r3"] = r3
            elif part == 10:   # dt matmuls: d = W4d^T (lg1 + lg2 + lg3), 3 passes
                dt_ps = sup[s]["dt"]
                lgs = (S["lg1"], S["lg2"], S["lg3"])
                for g in range(4):
                    gi = w * 4 + g
                    for li, lg in enumerate(lgs):
                        for k in range(2):
                            nc.tensor.matmul(dt_ps[:, gi],
                                             lg[:, k, g * 128:(g + 1) * 128],
                                             L4s[:, k],
                                             start=(li == 0 and k == 0),
                                             stop=(li == 2 and k == 1))

        def bw_a(t, i):
            S = st[t]
            v_ps = vps_pool.tile([128, 2, TN], F32, tag="vps", name=f"vps{i}")
            S.setdefault("v", {})[i] = v_ps
            for j in range(2):
                nc.tensor.matmul(v_ps[:, j],
                                 M3s[:, :, i, j * 128:(j + 1) * 128],
                                 S["r3"][:], start=True, stop=False,
                                 perf_mode=mybir.MatmulPerfMode.DoubleRow)
                nc.tensor.matmul(v_ps[:, j], Cbs[:, i, j * 128:(j + 1) * 128],
                                 ones[:], start=False, stop=False)

        def bw_b(t, i):
            S = st[t]
            v2s2 = sbv.tile([128, 2, TN], BF16, tag="v2s2", name=f"v2s2_{i}")
            S.setdefault("v2", {})[i] = v2s2
            nc.vector.tensor_tensor(v2s2[:], S["v"][i][:], S["s2"][:], OP.mult)

        def bw_c(t, i):
            S = st[t]
            for j in range(2):
                for k in range(2):
                    nc.tensor.matmul(S["v"][i][:, j],
                                     W2s[:, k, j * 128:(j + 1) * 128],
                                     S["v2"][i][:, k], start=False, stop=(k == 1))

        def bw_d(t, i):
            S = st[t]
            v1s1 = sbv.tile([128, 2, TN], BF16, tag="v1s1", name=f"v1s1_{i}")
            S.setdefault("v1", {})[i] = v1s1
            nc.vector.tensor_tensor(v1s1[:], S["v"][i][:], S["s1"][:], OP.mult)

        def bw_e(t, i):
            S = st[t]
            w = t % SUP
            jt_ps = sup[t // SUP]["jt"]
            for g in range(4):
                gi = w * 4 + g
                for k in range(2):
                    nc.tensor.matmul(jt_ps[:, gi, i * NS:(i + 1) * NS],
                                     S["v1"][i][:, k, g * 128:(g + 1) * 128],
                                     W1s[:, k], start=(k == 0), stop=(k == 1))

        def emit_copies(t, part):
            # per-supertile: copy jt/dt accumulators out of PSUM (Act).
            # part 0 (dtg, small) early: dt(td) of the next supertile waits
            # on it.  part 1 (jtg) at the Act stream end: its consumers
            # (bw_e alloc, prods) lag a full iteration.
            w, s = t % SUP, t // SUP
            if w != SUP - 1:
                return
            sq = s % SOLVE_SUPS
            if part == 0:
                if sq == 0:
                    grp["msb"] = mpool.tile([128, 10, SROWS], F32, tag="msb", name="msb")
                    grp["vsb"] = mpool.tile([128, 4, SROWS], F32, tag="vsb", name="vsb")
                    grp["csb"] = mpool.tile([128, SROWS, 4], F32, tag="csb", name="csb")
                    grp["jtg"] = jpool.tile([128, SROWS, 4 * NS], BF16, tag="jtg", name="jtg")
                    grp["dtg"] = jpool.tile([128, SROWS, NS], F32, tag="dtg", name="dtg")
                    grp["nsup"] = 0
                rows = slice(sq * GRPS, (sq + 1) * GRPS)
                nc.scalar.copy(grp["dtg"][:, rows], sup[s]["dt"][:])
                grp["nsup"] += 1
                sup[s]["jtg"] = grp["jtg"]
                sup[s]["dtg"] = grp["dtg"]
                sup[s]["msb"] = grp["msb"]
                sup[s]["vsb"] = grp["vsb"]
                sup[s]["csb"] = grp["csb"]
                sup[s]["sq"] = sq
                sup[s]["nsup"] = grp["nsup"]
            else:
                rows = slice(sq * GRPS, (sq + 1) * GRPS)
                nc.scalar.copy(sup[s]["jtg"][:, rows], sup[s]["jt"][:])

        def emit_prods(t):
            # deferred products for the supertile ending at tile t
            w, s = t % SUP, t // SUP
            if w != SUP - 1:
                return
            sq = sup[s]["sq"]
            rows = slice(sq * GRPS, (sq + 1) * GRPS)
            jts = sup[s]["jtg"][:, rows]
            dts = sup[s]["dtg"][:, rows]
            gprod = sb.tile([128, 10, GRPS, NS], BF16, tag="gprod")
            pbase = 0
            for a in range(4):
                nb = 4 - a
                in0 = jts[:, :, a * NS:(a + 1) * NS].unsqueeze(1) \
                    .to_broadcast((128, nb, GRPS, NS))
                in1 = jts[:, :, a * NS:].rearrange("p g (b n) -> p b g n", b=nb)
                nc.gpsimd.tensor_tensor(gprod[:, pbase:pbase + nb],
                                        in0, in1, OP.mult)
                pbase += nb
            vprod = sb.tile([128, 4, GRPS, NS], F32, tag="vprod")
            nc.gpsimd.tensor_tensor(
                vprod[:],
                jts[:].rearrange("p g (b n) -> p b g n", b=4),
                dts[:].unsqueeze(1).to_broadcast((128, 4, GRPS, NS)),
                OP.mult)
            sup[s]["gprod"] = gprod
            sup[s]["vprod"] = vprod

        def emit_reds(t):
            # deferred reductions for the supertile ending at tile t
            w, s = t % SUP, t // SUP
            if w != SUP - 1:
                return
            sq = sup[s]["sq"]
            rows = slice(sq * GRPS, (sq + 1) * GRPS)
            nc.vector.tensor_reduce(
                sup[s]["msb"][:, :, rows].unsqueeze(3), sup[s]["gprod"][:],
                AX.X, OP.add)
            nc.vector.tensor_reduce(
                sup[s]["vsb"][:, :, rows].unsqueeze(3), sup[s]["vprod"][:],
                AX.X, OP.add)
            if sup[s]["sq"] == SOLVE_SUPS - 1 or t == ntiles - 1:
                enqueue_solve(s, last=(t >= ntiles - SUP))

        def enqueue_solve(s, last=False):
            msb, vsb, csb = sup[s]["msb"], sup[s]["vsb"], sup[s]["csb"]
            jtg, dtg = sup[s]["jtg"], sup[s]["dtg"]
            nsup = sup[s]["nsup"]
            R = nsup * GRPS

            def m(i_):
                return msb[:, i_, :R].unsqueeze(2)

            def vv(i_):
                return vsb[:, i_, :R].unsqueeze(2)

            tt = {}

            def tmp(name):
                if name not in tt:
                    tt[name] = mpool.tile([128, SROWS, 1], F32, tag=f"t_{name}", name=f"t_{name}")
                return tt[name][:, :R]

            # final group: nothing left to overlap, so run the serial solve
            # chain on DVE (faster per-op) and split combine across engines.
            V = nc.vector if last else nc.gpsimd
            step_cost = 64 * 1.39 + 95
            k_now = cur_k[0]

            def q(*ops, lag=1):
                for op in ops:
                    pool_q.append((step_cost, k_now + lag, op))

            def mul(o, x, y):
                return lambda: V.tensor_tensor(o, x, y, OP.mult)

            def sub(o, x, y):
                return lambda: V.tensor_tensor(o, x, y, OP.subtract)

            def add(o, x, y):
                return lambda: V.tensor_tensor(o, x, y, OP.add)

            # lazily build AP closures (tiles exist already)
            m00, m01, m02, m03, m11, m12, m13, m22, m23, m33 = (m(i_) for i_ in range(10))
            x1, x2 = tmp("x1"), tmp("x2")
            detA, u10, u11 = tmp("detA"), tmp("u10"), tmp("u11")
            P00, P01, P10, P11 = tmp("P00"), tmp("P01"), tmp("P10"), tmp("P11")
            S00, S01, S11 = tmp("S00"), tmp("S01"), tmp("S11")
            w0, w1, detS = tmp("w0"), tmp("w1"), tmp("detS")
            cw2, cw3, q0, q1 = tmp("cw2"), tmp("cw3"), tmp("q0"), tmp("q1")
            dAS, rAS, rS = tmp("dAS"), tmp("rAS"), tmp("rS")
            x3, x4 = tmp("x3"), tmp("x4")

            q(mul(x1, m00, m11), mul(x2, m01, m01), sub(detA, x1, x2))
            q(mul(x1, m11, vv(0)), mul(x2, m01, vv(1)), sub(u10, x1, x2))
            q(mul(x1, m00, vv(1)), mul(x2, m01, vv(0)), sub(u11, x1, x2))
            q(mul(x1, m11, m02), mul(x2, m01, m12), sub(P00, x1, x2))
            q(mul(x1, m11, m03), mul(x2, m01, m13), sub(P01, x1, x2))
            q(mul(x1, m00, m12), mul(x2, m01, m02), sub(P10, x1, x2))
            q(mul(x1, m00, m13), mul(x2, m01, m03), sub(P11, x1, x2))
            q(mul(x1, m02, P00), mul(x2, m12, P10), add(x1, x1, x2))
            q(mul(S00, detA, m22), sub(S00, S00, x1))
            q(mul(x3, m02, P01), mul(x4, m12, P11), add(x3, x3, x4))
            q(mul(S01, detA, m23), sub(S01, S01, x3))
            q(mul(x1, m03, P01), mul(x2, m13, P11), add(x1, x1, x2))
            q(mul(S11, detA, m33), sub(S11, S11, x1))
            q(mul(x3, m02, u10), mul(x4, m12, u11), add(x3, x3, x4))
            q(mul(w0, detA, vv(2)), sub(w0, w0, x3))
            q(mul(x1, m03, u10), mul(x2, m13, u11), add(x1, x1, x2))
            q(mul(w1, detA, vv(3)), sub(w1, w1, x1))
            q(mul(x3, S00, S11), mul(x4, S01, S01), sub(detS, x3, x4))
            q(mul(x1, S11, w0), mul(x2, S01, w1), sub(cw2, x1, x2))
            q(mul(x3, S00, w1), mul(x4, S01, w0), sub(cw3, x3, x4))
            q(mul(x1, P00, cw2), mul(x2, P01, cw3), add(x1, x1, x2))
            q(mul(q0, u10, detS), sub(q0, q0, x1))
            q(mul(x3, P10, cw2), mul(x4, P11, cw3), add(x3, x3, x4))
            q(mul(q1, u11, detS), sub(q1, q1, x3))
            q(mul(dAS, detA, detS))
            q(lambda: nc.vector.reciprocal(rAS, dAS), lag=5)
            q(lambda: nc.vector.reciprocal(rS, detS), lag=5)
            q(mul(csb[:, :R, 0:1], q0, rAS), mul(csb[:, :R, 1:2], q1, rAS), lag=5)
            q(mul(csb[:, :R, 2:3], cw2, rS), mul(csb[:, :R, 3:4], cw3, rS), lag=5)

            # combine + output DMA on DVE, sliced into quarter-row chunks
            s_base = s - (nsup - 1)
            R2 = nsup * GRPS
            acc = sb.tile([128, SROWS, NS], F32, tag="acc")
            ctmp = sb.tile([128, SROWS, NS], F32, tag="ctmp")
            RQ = R2 // 4
            slice_cost = RQ * NS * 1.99 + 95

            def comb(a, r0, r1, E):
                def fn():
                    cb = csb[:, r0:r1, a:a + 1].to_broadcast((128, r1 - r0, NS))
                    E.tensor_tensor(ctmp[:, r0:r1], cb,
                                    jtg[:, r0:r1, a * NS:(a + 1) * NS], OP.mult)
                    if a == 0:
                        E.tensor_tensor(acc[:, r0:r1], dtg[:, r0:r1],
                                        ctmp[:, r0:r1], OP.subtract)
                    else:
                        E.tensor_tensor(acc[:, r0:r1], acc[:, r0:r1],
                                        ctmp[:, r0:r1], OP.subtract)
                return fn

            for a in range(4):
                for rq in range(4):
                    # last group: alternate combine slices across Pool and DVE
                    E = (nc.gpsimd if rq % 2 else nc.vector) if last \
                        else nc.gpsimd
                    pool_q.append((2 * slice_cost, k_now + 5 + a,
                                   comb(a, rq * RQ, (rq + 1) * RQ, E)))

            def out_dma():
                eb = s_base * SUP * TN
                nc.sync.dma_start(
                    out_d[eb:eb + R2 * 128, :].rearrange("(g p) m -> p g m", p=128),
                    acc[:, :R2])

            pool_q.append((0.0, k_now + 10, out_dma))

        # ---------------- the scheduled iterations
        for k in range(ntiles + 4):
            tb = k - 1           # backward tile (stages a-d)
            tf = k               # forward tile
            te = k - 2           # deferred stage_e / copies tile
            td = k - 1           # deferred dt tile
            tp = k - 3           # deferred postproc tile (prods/reduces)
            fw = tf < ntiles
            bw = 0 <= tb < ntiles
            ee = 0 <= te < ntiles
            dd = 0 <= td < ntiles
            pp = 0 <= tp < ntiles

            if fw: emit_fwd(tf, 0)                      # PE: a1 (+dma)
            if bw: bw_a(tb, 0); bw_a(tb, 1)             # PE: A0 A1
            if fw: emit_fwd(tf, 1)                      # Act: e1 lg1
            if ee:
                bw_e(te, 0); bw_e(te, 1)                # PE (deferred)
            if bw: bw_b(tb, 0); bw_b(tb, 1)             # DVE: B0 B1
            if fw: emit_fwd(tf, 2)                      # Act: r1 ; Pool: sg1
            if ee: emit_copies(te, 0)                   # Act: dtg copy (small)
            if bw: bw_c(tb, 0)                          # PE: C0
            if fw: emit_fwd(tf, 3)                      # PE: a2
            if bw: bw_d(tb, 0)                          # DVE: D0
            if ee:
                bw_e(te, 2); bw_e(te, 3)                # PE (deferred, D2/D3 late)
            if bw: bw_c(tb, 1)                          # PE: C1
            if dd: emit_fwd(td, 10)                     # PE: dt (deferred)
            if bw: bw_d(tb, 1)                          # DVE: D1
            if bw: bw_a(tb, 2)                          # PE: A2 (after D0)
            if fw: emit_fwd(tf, 4)                      # Act: e2 lg2 r2
            if fw: emit_fwd(tf, 71)                     # PE: a3 lg1-half
            if fw: emit_fwd(tf, 6)                      # Pool: sg2
            if bw: bw_b(tb, 2)                          # DVE: B2
            if pp: emit_prods(tp)                       # Pool: gram/vprod
            if fw: emit_fwd(tf, 72)                     # PE: a3 lg2-half
            if bw: bw_a(tb, 3)                          # PE: A3 (after D1)
            if bw: bw_b(tb, 3)                          # DVE: B3
            if bw: bw_c(tb, 2)                          # PE: C2
            if bw: bw_d(tb, 2)                          # DVE: D2
            if bw: bw_c(tb, 3)                          # PE: C3
            if bw: bw_d(tb, 3)                          # DVE: D3
            if pp: emit_reds(tp)                        # DVE (+ enqueue solve)
            if fw: emit_fwd(tf, 8)                      # Act: e3 lg3 r3
            if ee: emit_copies(te, 1)                   # Act: jtg copy (late)
            cur_k[0] = k
            drain_pool_q(3600)
            if te - 1 >= 0:
                st.pop(te - 1, None)

        while pool_q:
            cost, mink, fn = pool_q.popleft()
            fn()

    _split_multi_waits(nc)
    return nc


# ---------------------------------------------------------------- entry point
def kernel(zstates, W1, W2, W3, W4):
    from concourse.bass_utils import run_bass_kernel_spmd

    key = "full"
    if key not in _cache:
        _cache[key] = _build()
    nc = _cache[key]

    wm = _prep_weights(W1, W2, W3, W4)
    z = np.asarray(zstates, np.float32).reshape(NCORES, BP, NS)
    in_maps = [
        {**wm, "zT": np.ascontiguousarray(z[c].T)} for c in range(NCORES)
    ]
    res = run_bass_kernel_spmd(nc, in_maps, core_ids=list(range(NCORES)))
    return np.concatenate([res.results[c]["out"] for c in range(NCORES)], axis=0)
